# revision 26
# baseline (speedup 1.0000x reference)
"""Trainium2 Bass kernel for nn_MultiHeadMambaBlock_79130477461819.

Sharding: 8 cores = 4 batches x 2 halves of d_inner (tensor parallel over
scan channels). x_proj / out_proj contract over full d_inner -> paired
AllReduce. Selective scan uses the DVE linear-recurrence instruction
(tensor_tensor_scan: state = dA*state + dBu along free dim), 128
channels/partitions per instruction, one scan per (d-tile, state index n).
"""
import sys, os
sys.path.insert(0, "/opt/trn_rl_repo")
os.environ.setdefault("JAX_PLATFORMS", "axon,cpu")

import numpy as np

DIM = 512
D_INNER = 1024
DSH = 512          # d_inner shard per core
N = 16             # d_state
D_CONV = 4
DT_RANK = 32
D_GEOM = 256
B, L = 4, 2048
EPS = 1e-5
TCH = 512
NCH = L // TCH     # 4 chunks
NT = L // 128      # 16 t-tiles
NDT = DSH // 128   # 4 d-tiles
NCT = DIM // 128   # 4 c-tiles
MM_FR = True       # float32r fast matmuls
SIM1 = False       # build without collectives for TimelineSim

_cache = {}


def _build(sim1=False):
    import concourse.mybir as mybir
    import concourse.bacc as bacc
    import concourse.tile as tile

    F = mybir.dt.float32
    FR = mybir.dt.float32r if MM_FR else F
    AF = mybir.ActivationFunctionType
    ALU = mybir.AluOpType
    AX = mybir.AxisListType

    nc = bacc.Bacc("TRN2", target_bir_lowering=False, debug=False,
                   num_devices=1 if sim1 else 8)

    def din(name, shape, dt=F):
        return nc.dram_tensor(name, list(shape), dt, kind="ExternalInput").ap()

    x_d = din("x", [L, DIM])                      # this core's batch, t-major
    i128_d = din("i128", [128, 128], FR)
    winT_d = din("winT", [DIM, 2 * DSH], FR)      # [c, xm|z]
    nw_d = din("nw", [128, NCT]); nb_d = din("nb", [128, NCT])
    convw_d = din("convw", [128, NDT * D_CONV]); convb_d = din("convb", [128, NDT])
    xpT_d = din("xpT", [DSH, 64], FR)
    dtT_d = din("dtT", [DT_RANK, DSH], FR)
    dtb_d = din("dtb", [128, NDT])
    acols_d = din("acols", [128, NDT * N])        # A = -exp(A_log)
    dcol_d = din("dcol", [128, NDT])              # D vec
    woutT_d = din("woutT", [DSH, DIM], FR)
    wmT_d = din("wmT", [DIM, DIM], FR)
    mb_d = din("mb", [128, NCT])
    wgT_d = din("wgT", [DIM, D_GEOM], FR)
    gb_d = din("gb", [128, D_GEOM // 128])
    nmw_d = din("nmw", [1, DIM]); nmb_d = din("nmb", [1, DIM])
    ngw_d = din("ngw", [1, D_GEOM]); ngb_d = din("ngb", [1, D_GEOM])
    ym_d = nc.dram_tensor("ym", [L, DIM], F, kind="ExternalOutput").ap()
    yg_d = nc.dram_tensor("yg", [L, D_GEOM], F, kind="ExternalOutput").ap()

    with tile.TileContext(nc) as tc:
        with tc.tile_pool(name="const", bufs=1) as cp, \
             tc.tile_pool(name="dram", bufs=1, space="DRAM") as dp:

            # ------- constants -------
            I128 = cp.tile([128, 128], FR); nc.sync.dma_start(I128[:], i128_d)
            nw = cp.tile([128, NCT], F); nc.sync.dma_start(nw[:], nw_d)
            nb = cp.tile([128, NCT], F); nc.sync.dma_start(nb[:], nb_d)
            convw = cp.tile([128, NDT * D_CONV], F); nc.sync.dma_start(convw[:], convw_d)
            convb = cp.tile([128, NDT], F); nc.sync.dma_start(convb[:], convb_d)
            dtb = cp.tile([128, NDT], F); nc.sync.dma_start(dtb[:], dtb_d)
            acols = cp.tile([128, NDT * N], F); nc.sync.dma_start(acols[:], acols_d)
            dcol = cp.tile([128, NDT], F); nc.sync.dma_start(dcol[:], dcol_d)
            mb = cp.tile([128, NCT], F); nc.sync.dma_start(mb[:], mb_d)
            gb = cp.tile([128, D_GEOM // 128], F); nc.sync.dma_start(gb[:], gb_d)
            xpT = cp.tile([128, NDT * 64], FR)
            for j in range(NDT):
                nc.sync.dma_start(xpT[:, j * 64:(j + 1) * 64], xpT_d[j * 128:(j + 1) * 128, :])
            dtT = cp.tile([DT_RANK, DSH], FR); nc.sync.dma_start(dtT[:], dtT_d)
            woutT = cp.tile([128, NDT * DIM], FR)
            for j in range(NDT):
                nc.sync.dma_start(woutT[:, j * DIM:(j + 1) * DIM], woutT_d[j * 128:(j + 1) * 128, :])
            states = cp.tile([128, NDT * N], F)

            # ------- DRAM scratch -------
            xm_dram = dp.tile([DSH, L + 4], F)
            z_dram = dp.tile([DSH, L], F)
            u_dram = dp.tile([DSH, L], F)
            xdblp_dram = dp.tile([NCH * 64, TCH], F)
            xdbl_dram = dp.tile([NCH * 64, TCH], F)
            hmp_dram = dp.tile([DIM, L], F)
            hm_dram = dp.tile([DIM, L], F)

            zpad = cp.tile([128, 4], F)
            nc.vector.memset(zpad[:], 0.0)
            epsc = cp.tile([128, 1], F)
            nc.vector.memset(epsc[:], EPS)
            for j in range(NDT):
                nc.sync.dma_start(xm_dram[j * 128:(j + 1) * 128, 0:4], zpad[:])

            # ======= phase A: LN(x)->hT, in_proj, conv, silu, x_proj =======
            with tc.tile_pool(name="hp1", bufs=1) as hp1, \
                 tc.tile_pool(name="hp2", bufs=2) as hp2, \
                 tc.tile_pool(name="ppA", bufs=2, space="PSUM") as ppA:
                hT = [hp1.tile([128, L], FR, tag=f"hT{ct}", name=f"hT{ct}") for ct in range(NCT)]
                winT = []
                for kt in range(NCT):
                    wt_ = hp1.tile([128, 2 * DSH], FR, tag=f"winT{kt}", name=f"winT{kt}")
                    nc.sync.dma_start(wt_[:], winT_d[kt * 128:(kt + 1) * 128, :])
                    winT.append(wt_)
                for tt in range(NT):
                    xt = hp2.tile([128, DIM], F, tag="xt")
                    nc.sync.dma_start(xt[:], x_d[tt * 128:(tt + 1) * 128, :])
                    sm = hp2.tile([128, 1], F, tag="sm")
                    nc.vector.reduce_sum(sm[:], xt[:], axis=AX.X)
                    mu = hp2.tile([128, 1], F, tag="mu")
                    nc.scalar.mul(mu[:], sm[:], 1.0 / DIM)
                    cen = hp2.tile([128, DIM], FR, tag="cen")
                    nc.vector.tensor_scalar_sub(cen[:], xt[:], mu[:])
                    sq = hp2.tile([128, DIM], F, tag="sq")
                    vs = hp2.tile([128, 1], F, tag="vs")
                    nc.scalar.activation(sq[:], cen[:], AF.Square, accum_out=vs[:])
                    sd = hp2.tile([128, 1], F, tag="sd")
                    nc.scalar.activation(sd[:], vs[:], AF.Sqrt, bias=epsc[:], scale=1.0 / DIM)
                    rstd = hp2.tile([128, 1], F, tag="rstd")
                    nc.vector.reciprocal(rstd[:], sd[:])
                    Dg = hp2.tile([128, 128], FR, tag="Dg")
                    nc.vector.tensor_scalar_mul(Dg[:], I128[:], rstd[:])
                    for ct in range(NCT):
                        ph = ppA.tile([128, 128], F, tag="ph")
                        nc.tensor.matmul(ph[:], cen[:, ct * 128:(ct + 1) * 128], Dg[:],
                                         start=True, stop=True)
                        nc.vector.scalar_tensor_tensor(
                            hT[ct][:, tt * 128:(tt + 1) * 128], ph[:],
                            nw[:, ct:ct + 1],
                            nb[:, ct:ct + 1].broadcast_to([128, 128]),
                            ALU.mult, ALU.add)

                for ch in range(NCH):
                    c0 = ch * TCH
                    for half in range(2):          # 0: xm, 1: z
                        for mt in range(NDT):
                            px = ppA.tile([128, TCH], F, tag="px")
                            for kt in range(NCT):
                                nc.tensor.matmul(
                                    px[:],
                                    winT[kt][:, half * DSH + mt * 128: half * DSH + (mt + 1) * 128],
                                    hT[kt][:, c0:c0 + TCH],
                                    start=(kt == 0), stop=(kt == NCT - 1))
                            if half == 0:
                                xms = hp2.tile([128, TCH], F, tag="xms")
                                nc.scalar.copy(xms[:], px[:])
                                nc.sync.dma_start(
                                    xm_dram[mt * 128:(mt + 1) * 128, 4 + c0:4 + c0 + TCH], xms[:])
                            else:
                                zs = hp2.tile([128, TCH], F, tag="zs")
                                nc.scalar.activation(zs[:], px[:], AF.Silu)
                                nc.sync.dma_start(
                                    z_dram[mt * 128:(mt + 1) * 128, c0:c0 + TCH], zs[:])
                    pxp = ppA.tile([64, TCH], F, tag="pxp")
                    for j in range(NDT):
                        xmc = hp2.tile([128, TCH + 4], F, tag="xmc")
                        nc.sync.dma_start(xmc[:], xm_dram[j * 128:(j + 1) * 128, c0:c0 + TCH + 4])
                        acc = hp2.tile([128, TCH], F, tag="acc")
                        nc.vector.scalar_tensor_tensor(
                            acc[:], xmc[:, 1:1 + TCH], convw[:, j * 4:j * 4 + 1],
                            convb[:, j:j + 1].broadcast_to([128, TCH]),
                            ALU.mult, ALU.add)
                        acc2 = hp2.tile([128, TCH], F, tag="acc2")
                        nc.vector.scalar_tensor_tensor(
                            acc2[:], xmc[:, 2:2 + TCH], convw[:, j * 4 + 1:j * 4 + 2],
                            acc[:], ALU.mult, ALU.add)
                        nc.vector.scalar_tensor_tensor(
                            acc[:], xmc[:, 3:3 + TCH], convw[:, j * 4 + 2:j * 4 + 3],
                            acc2[:], ALU.mult, ALU.add)
                        nc.vector.scalar_tensor_tensor(
                            acc2[:], xmc[:, 4:4 + TCH], convw[:, j * 4 + 3:j * 4 + 4],
                            acc[:], ALU.mult, ALU.add)
                        uc = hp2.tile([128, TCH], F, tag="uc")
                        nc.scalar.activation(uc[:], acc2[:], AF.Silu)
                        nc.sync.dma_start(u_dram[j * 128:(j + 1) * 128, c0:c0 + TCH], uc[:])
                        ucr = hp2.tile([128, TCH], FR, tag="ucr")
                        nc.vector.tensor_copy(ucr[:], uc[:])
                        nc.tensor.matmul(pxp[:], xpT[:, j * 64:(j + 1) * 64], ucr[:],
                                         start=(j == 0), stop=(j == NDT - 1))
                    xps = hp2.tile([64, TCH], F, tag="xps")
                    nc.scalar.copy(xps[:], pxp[:])
                    nc.sync.dma_start(xdblp_dram[ch * 64:(ch + 1) * 64, :], xps[:])
                    if sim1:
                        nc.sync.dma_start(xdbl_dram[ch * 64:(ch + 1) * 64, :],
                                          xdblp_dram[ch * 64:(ch + 1) * 64, :])
                    else:
                        nc.gpsimd.collective_compute(
                            "AllReduce", ALU.add,
                            replica_groups=[[0, 1], [2, 3], [4, 5], [6, 7]],
                            ins=[xdblp_dram[ch * 64:(ch + 1) * 64, :]],
                            outs=[xdbl_dram[ch * 64:(ch + 1) * 64, :]])

            # ------- AllReduce x_dbl over the d_inner pair -------
            # ======= phase B: dt_proj, scan, gate, out_proj partial =======
            with tc.tile_pool(name="sp1", bufs=1) as sp1, \
                 tc.tile_pool(name="sp2", bufs=2) as sp2, \
                 tc.tile_pool(name="ppB", bufs=2, space="PSUM") as ppB:
                for ch in range(NCH):
                    c0 = ch * TCH
                    r0 = ch * 64
                    dtTf = sp2.tile([DT_RANK, TCH], F, tag="dtTf")
                    nc.sync.dma_start(dtTf[:], xdbl_dram[r0:r0 + DT_RANK, :])
                    dtTr = sp2.tile([DT_RANK, TCH], FR, tag="dtTr")
                    nc.scalar.copy(dtTr[:], dtTf[:])
                    brep = sp1.tile([128, N, TCH], F, tag="brep")
                    nc.sync.dma_start(
                        brep[:],
                        xdbl_dram[r0 + DT_RANK:r0 + DT_RANK + N, :].partition_broadcast(128))
                    crep = sp1.tile([128, N, TCH], F, tag="crep")
                    nc.sync.dma_start(
                        crep[:],
                        xdbl_dram[r0 + DT_RANK + N:r0 + DT_RANK + 2 * N, :].partition_broadcast(128))
                    y2 = []
                    for j in range(NDT):
                        pd = ppB.tile([128, TCH], F, tag="pd")
                        nc.tensor.matmul(pd[:], dtT[:, j * 128:(j + 1) * 128],
                                         dtTr[:], start=True, stop=True)
                        expd = sp2.tile([128, TCH], F, tag="expd")
                        nc.scalar.activation(expd[:], pd[:], AF.Exp, bias=dtb[:, j:j + 1])
                        delta = sp2.tile([128, TCH], F, tag="delta")
                        nc.scalar.activation(delta[:], expd[:], AF.Ln, bias=1.0)
                        uc2 = sp2.tile([128, TCH], F, tag="uc2")
                        nc.sync.dma_start(uc2[:], u_dram[j * 128:(j + 1) * 128, c0:c0 + TCH])
                        du = sp2.tile([128, TCH], F, tag="du")
                        nc.gpsimd.tensor_mul(du[:], delta[:], uc2[:])
                        ht = sp1.tile([128, N * TCH], F, tag="ht")
                        ht_nt = ht[:].rearrange("p (t n) -> p n t", n=N)   # n minor in memory
                        for n in range(N):
                            dA = sp2.tile([128, TCH], F, tag="dA", bufs=3)
                            nc.scalar.activation(dA[:], delta[:], AF.Exp,
                                                 scale=acols[:, j * N + n:j * N + n + 1])
                            dBu = sp2.tile([128, TCH], F, tag=f"dBu{n % 2}", bufs=3)
                            eng = nc.vector if n % 8 < 3 else nc.gpsimd
                            eng.tensor_mul(dBu[:], du[:], brep[:, n, :])
                            init = 0.0 if ch == 0 else states[:, j * N + n:j * N + n + 1]
                            nc.vector.tensor_tensor_scan(
                                ht_nt[:, n], dA[:], dBu[:], init,
                                ALU.mult, ALU.add)
                            if ch < NCH - 1:
                                nc.vector.tensor_copy(
                                    states[:, j * N + n:j * N + n + 1],
                                    ht[:, (TCH - 1) * N + n:(TCH - 1) * N + n + 1])
                        nh = 6
                        nc.vector.tensor_mul(ht_nt[:, :nh], ht_nt[:, :nh], crep[:, :nh])
                        nc.gpsimd.tensor_mul(ht_nt[:, nh:], ht_nt[:, nh:], crep[:, nh:])
                        yv = sp2.tile([128, TCH], F, tag="yv")
                        nc.vector.tensor_reduce(yv[:], ht[:].rearrange("p (t n) -> p t n", n=N),
                                                AX.X, ALU.add)
                        ys = sp2.tile([128, TCH], F, tag="ys")
                        nc.vector.scalar_tensor_tensor(ys[:], uc2[:], dcol[:, j:j + 1], yv[:],
                                                       ALU.mult, ALU.add)
                        zc = sp2.tile([128, TCH], F, tag="zc")
                        nc.sync.dma_start(zc[:], z_dram[j * 128:(j + 1) * 128, c0:c0 + TCH])
                        y2j = sp2.tile([128, TCH], FR, tag=f"y2_{j}", name=f"y2_{j}")
                        nc.vector.tensor_mul(y2j[:], ys[:], zc[:])
                        y2.append(y2j)
                    for mt in range(NCT):
                        po = ppB.tile([128, TCH], F, tag="po")
                        for j in range(NDT):
                            nc.tensor.matmul(
                                po[:], woutT[:, j * DIM + mt * 128: j * DIM + (mt + 1) * 128],
                                y2[j][:], start=(j == 0), stop=(j == NDT - 1))
                        hms = sp2.tile([128, TCH], F, tag="hms")
                        nc.scalar.copy(hms[:], po[:])
                        nc.sync.dma_start(hmp_dram[mt * 128:(mt + 1) * 128, c0:c0 + TCH], hms[:])

            # ------- AllReduce out_proj partial over the pair -------
            if sim1:
                nc.sync.dma_start(hm_dram.opt(), hmp_dram.opt())
            else:
                nc.gpsimd.collective_compute(
                    "AllReduce", ALU.add,
                    replica_groups=[[0, 1], [2, 3], [4, 5], [6, 7]],
                    ins=[hmp_dram.opt()], outs=[hm_dram.opt()])

            # ======= phase C: match & geom heads + final LNs =======
            with tc.tile_pool(name="mp1", bufs=1) as mp1, \
                 tc.tile_pool(name="mp2", bufs=2) as mp2, \
                 tc.tile_pool(name="ppC", bufs=2, space="PSUM") as ppC:
                wmT = mp1.tile([128, NCT * DIM], FR)
                for j in range(NCT):
                    nc.sync.dma_start(wmT[:, j * DIM:(j + 1) * DIM], wmT_d[j * 128:(j + 1) * 128, :])
                wgT = mp1.tile([128, NCT * D_GEOM], FR)
                for j in range(NCT):
                    nc.sync.dma_start(wgT[:, j * D_GEOM:(j + 1) * D_GEOM], wgT_d[j * 128:(j + 1) * 128, :])
                nmw = mp1.tile([128, DIM], F); nc.sync.dma_start(nmw[:], nmw_d.broadcast_to([128, DIM]))
                nmb = mp1.tile([128, DIM], F); nc.sync.dma_start(nmb[:], nmb_d.broadcast_to([128, DIM]))
                ngw = mp1.tile([128, D_GEOM], F); nc.sync.dma_start(ngw[:], ngw_d.broadcast_to([128, D_GEOM]))
                ngb = mp1.tile([128, D_GEOM], F); nc.sync.dma_start(ngb[:], ngb_d.broadcast_to([128, D_GEOM]))
                hmT = []
                for ct in range(NCT):
                    t_ = mp1.tile([128, L], FR, tag=f"hmT{ct}", name=f"hmT{ct}")
                    tf = mp2.tile([128, L], F, tag="hmTf")
                    nc.sync.dma_start(tf[:], hm_dram[ct * 128:(ct + 1) * 128, :])
                    nc.scalar.copy(t_[:], tf[:])
                    hmT.append(t_)

                def layer_norm_store(src, width, w_rep, b_rep, out_ap):
                    sm2 = mp2.tile([128, 1], F, tag="sm2")
                    nc.vector.reduce_sum(sm2[:], src[:], axis=AX.X)
                    mu2 = mp2.tile([128, 1], F, tag="mu2")
                    nc.scalar.mul(mu2[:], sm2[:], 1.0 / width)
                    cen2 = mp2.tile([128, width], F, tag=f"cen2_{width}")
                    nc.vector.tensor_scalar_sub(cen2[:], src[:], mu2[:])
                    sq2 = mp2.tile([128, width], F, tag=f"sq2_{width}")
                    vs2 = mp2.tile([128, 1], F, tag="vs2")
                    nc.scalar.activation(sq2[:], cen2[:], AF.Square, accum_out=vs2[:])
                    sd2 = mp2.tile([128, 1], F, tag="sd2")
                    nc.scalar.activation(sd2[:], vs2[:], AF.Sqrt, bias=epsc[:], scale=1.0 / width)
                    rstd2 = mp2.tile([128, 1], F, tag="rstd2")
                    nc.vector.reciprocal(rstd2[:], sd2[:])
                    o1 = mp2.tile([128, width], F, tag=f"o1_{width}")
                    nc.vector.scalar_tensor_tensor(o1[:], cen2[:], rstd2[:], w_rep[:, :width],
                                                   ALU.mult, ALU.mult)
                    o2 = mp2.tile([128, width], F, tag=f"o2_{width}")
                    nc.vector.tensor_add(o2[:], o1[:], b_rep[:, :width])
                    nc.sync.dma_start(out_ap, o2[:])

                for tb in range(L // TCH):
                    b0 = tb * TCH
                    pms = []
                    for mt in range(NCT):
                        pm = ppC.tile([128, TCH], F, tag=f"pm{mt}", name=f"pm{mt}", bufs=1)
                        for kt in range(NCT):
                            nc.tensor.matmul(
                                pm[:], wmT[:, kt * DIM + mt * 128: kt * DIM + mt * 128 + 128],
                                hmT[kt][:, b0:b0 + TCH],
                                start=(kt == 0), stop=(kt == NCT - 1))
                        pms.append(pm)
                    pgs = []
                    for mt in range(D_GEOM // 128):
                        pg = ppC.tile([128, TCH], F, tag=f"pg{mt}", name=f"pg{mt}", bufs=1)
                        for kt in range(NCT):
                            nc.tensor.matmul(
                                pg[:], wgT[:, kt * D_GEOM + mt * 128: kt * D_GEOM + mt * 128 + 128],
                                hmT[kt][:, b0:b0 + TCH],
                                start=(kt == 0), stop=(kt == NCT - 1))
                        pgs.append(pg)
                    for st in range(TCH // 128):
                        t0 = b0 + st * 128
                        s0 = st * 128
                        mrow = mp2.tile([128, DIM], F, tag="mrow")
                        xres = mp2.tile([128, DIM], F, tag="xres")
                        nc.sync.dma_start(xres[:], x_d[t0:t0 + 128, :])
                        for mt in range(NCT):
                            mc = mp2.tile([128, 128], FR, tag="mc")
                            nc.vector.tensor_scalar_add(mc[:], pms[mt][:, s0:s0 + 128],
                                                        mb[:, mt:mt + 1])
                            pt = ppC.tile([128, 128], F, tag="pt", bufs=1)
                            nc.tensor.matmul(pt[:], mc[:], I128[:], start=True, stop=True)
                            nc.vector.tensor_add(mrow[:, mt * 128:(mt + 1) * 128], pt[:],
                                                 xres[:, mt * 128:(mt + 1) * 128])
                        layer_norm_store(mrow, DIM, nmw, nmb, ym_d[t0:t0 + 128, :])

                        grow = mp2.tile([128, D_GEOM], F, tag="grow")
                        for mt in range(D_GEOM // 128):
                            gc = mp2.tile([128, 128], FR, tag="gc")
                            nc.vector.tensor_scalar_add(gc[:], pgs[mt][:, s0:s0 + 128],
                                                        gb[:, mt:mt + 1])
                            pgt = ppC.tile([128, 128], F, tag="pgt", bufs=1)
                            nc.tensor.matmul(pgt[:], gc[:], I128[:], start=True, stop=True)
                            nc.vector.tensor_copy(grow[:, mt * 128:(mt + 1) * 128], pgt[:])
                        layer_norm_store(grow, D_GEOM, ngw, ngb, yg_d[t0:t0 + 128, :])

    nc.compile()
    return nc


def _host_inputs(inputs, core):
    b, s = core // 2, core % 2
    f32 = np.float32
    A = (-np.exp(np.asarray(inputs["A_log"], f32)))[s * DSH:(s + 1) * DSH]  # [512,16]
    in_proj_w = np.asarray(inputs["in_proj_w"], f32)
    winT = np.concatenate([in_proj_w[s * DSH:(s + 1) * DSH],
                           in_proj_w[D_INNER + s * DSH:D_INNER + (s + 1) * DSH]], 0).T
    conv_w = np.asarray(inputs["conv_w"], f32)[s * DSH:(s + 1) * DSH, 0, :]     # [512,4]
    conv_b = np.asarray(inputs["conv_b"], f32)[s * DSH:(s + 1) * DSH]
    xp = np.asarray(inputs["x_proj_w"], f32)[:, s * DSH:(s + 1) * DSH]          # [64, 512]
    dt_w = np.asarray(inputs["dt_proj_w"], f32)[s * DSH:(s + 1) * DSH]          # [512, 32]
    dt_b = np.asarray(inputs["dt_proj_b"], f32)[s * DSH:(s + 1) * DSH]
    Dv = np.asarray(inputs["D"], f32)[s * DSH:(s + 1) * DSH]
    wout = np.asarray(inputs["mix_out_w"], f32)[:, s * DSH:(s + 1) * DSH]       # [512, 512]

    def col128(v):        # [512] -> [128, k] (col j = slice for tile j)
        return np.ascontiguousarray(v.reshape(-1, 128).T)

    def grid128(m):       # [nt*128, k] -> [128, nt*k]
        nt = m.shape[0] // 128
        return np.ascontiguousarray(m.reshape(nt, 128, -1).transpose(1, 0, 2).reshape(128, -1))

    return {
        "x": np.ascontiguousarray(np.asarray(inputs["x"], f32)[b]),
        "i128": np.eye(128, dtype=f32),
        "winT": np.ascontiguousarray(winT),
        "nw": col128(np.asarray(inputs["norm_w"], f32)),
        "nb": col128(np.asarray(inputs["norm_b"], f32)),
        "convw": grid128(conv_w),
        "convb": col128(conv_b),
        "xpT": np.ascontiguousarray(xp.T),
        "dtT": np.ascontiguousarray(dt_w.T),
        "dtb": col128(dt_b),
        "acols": grid128(A),
        "dcol": col128(Dv),
        "woutT": np.ascontiguousarray(wout.T),
        "wmT": np.ascontiguousarray(np.asarray(inputs["match_w"], f32).T),
        "mb": col128(np.asarray(inputs["match_b"], f32)),
        "wgT": np.ascontiguousarray(np.asarray(inputs["geom_w"], f32).T),
        "gb": col128(np.asarray(inputs["geom_b"], f32)),
        "nmw": np.asarray(inputs["normm_w"], f32).reshape(1, -1),
        "nmb": np.asarray(inputs["normm_b"], f32).reshape(1, -1),
        "ngw": np.asarray(inputs["normg_w"], f32).reshape(1, -1),
        "ngb": np.asarray(inputs["normg_b"], f32).reshape(1, -1),
    }


def kernel(**inputs):
    from concourse import bass_utils
    if "nc" not in _cache:
        _cache["nc"] = _build()
    nc = _cache["nc"]
    in_maps = [_host_inputs(inputs, c) for c in range(8)]
    res = bass_utils.run_bass_kernel_spmd(nc, in_maps, core_ids=list(range(8)),
                                          trace=bool(os.environ.get("BASS_TRACE")))
    _cache["last"] = res
    if res.exec_time_ns is not None:
        print(f"HW exec time: {res.exec_time_ns} ns")
        if res.instructions_and_trace:
            print("trace:", res.instructions_and_trace[1])
    ym = np.zeros((B, L, DIM), np.float32)
    yg = np.zeros((B, L, D_GEOM), np.float32)
    for b in range(B):
        ym[b] = res.results[2 * b]["ym"]
        yg[b] = res.results[2 * b]["yg"]
    return ym, yg


# revision 27
# speedup vs baseline: 1.0198x; 1.0198x over previous
"""Trainium2 Bass kernel for nn_MultiHeadMambaBlock_79130477461819.

Sharding: 8 cores = 4 batches x 2 halves of d_inner (tensor parallel over
scan channels). x_proj / out_proj contract over full d_inner -> paired
AllReduce. Selective scan uses the DVE linear-recurrence instruction
(tensor_tensor_scan: state = dA*state + dBu along free dim), 128
channels/partitions per instruction, one scan per (d-tile, state index n).
"""
import sys, os
sys.path.insert(0, "/opt/trn_rl_repo")
os.environ.setdefault("JAX_PLATFORMS", "axon,cpu")

import numpy as np

DIM = 512
D_INNER = 1024
DSH = 512          # d_inner shard per core
N = 16             # d_state
D_CONV = 4
DT_RANK = 32
D_GEOM = 256
B, L = 4, 2048
EPS = 1e-5
TCH = 512
NCH = L // TCH     # 4 chunks
NT = L // 128      # 16 t-tiles
NDT = DSH // 128   # 4 d-tiles
NCT = DIM // 128   # 4 c-tiles
MM_FR = True       # float32r fast matmuls
SIM1 = False       # build without collectives for TimelineSim

_cache = {}


def _build(sim1=False):
    import concourse.mybir as mybir
    import concourse.bacc as bacc
    import concourse.tile as tile

    F = mybir.dt.float32
    FR = mybir.dt.float32r if MM_FR else F
    AF = mybir.ActivationFunctionType
    ALU = mybir.AluOpType
    AX = mybir.AxisListType

    nc = bacc.Bacc("TRN2", target_bir_lowering=False, debug=False,
                   num_devices=1 if sim1 else 8)

    def din(name, shape, dt=F):
        return nc.dram_tensor(name, list(shape), dt, kind="ExternalInput").ap()

    x_d = din("x", [L, DIM])                      # this core's batch, t-major
    i128_d = din("i128", [128, 128], FR)
    winT_d = din("winT", [DIM, 2 * DSH], FR)      # [c, xm|z]
    nw_d = din("nw", [128, NCT]); nb_d = din("nb", [128, NCT])
    convw_d = din("convw", [128, NDT * D_CONV]); convb_d = din("convb", [128, NDT])
    xpT_d = din("xpT", [DSH, 64], FR)
    dtT_d = din("dtT", [DT_RANK, DSH], FR)
    dtb_d = din("dtb", [128, NDT])
    acols_d = din("acols", [128, NDT * N])        # A = -exp(A_log)
    dcol_d = din("dcol", [128, NDT])              # D vec
    woutT_d = din("woutT", [DSH, DIM], FR)
    wmT_d = din("wmT", [DIM, DIM], FR)
    mb_d = din("mb", [128, NCT])
    wgT_d = din("wgT", [DIM, D_GEOM], FR)
    gb_d = din("gb", [128, D_GEOM // 128])
    nmw_d = din("nmw", [1, DIM]); nmb_d = din("nmb", [1, DIM])
    ngw_d = din("ngw", [1, D_GEOM]); ngb_d = din("ngb", [1, D_GEOM])
    ym_d = nc.dram_tensor("ym", [L, DIM], F, kind="ExternalOutput").ap()
    yg_d = nc.dram_tensor("yg", [L, D_GEOM], F, kind="ExternalOutput").ap()

    with tile.TileContext(nc) as tc:
        with tc.tile_pool(name="const", bufs=1) as cp, \
             tc.tile_pool(name="dram", bufs=1, space="DRAM") as dp:

            # ------- constants -------
            I128 = cp.tile([128, 128], FR); nc.sync.dma_start(I128[:], i128_d)
            nw = cp.tile([128, NCT], F); nc.sync.dma_start(nw[:], nw_d)
            nb = cp.tile([128, NCT], F); nc.sync.dma_start(nb[:], nb_d)
            convw = cp.tile([128, NDT * D_CONV], F); nc.sync.dma_start(convw[:], convw_d)
            convb = cp.tile([128, NDT], F); nc.sync.dma_start(convb[:], convb_d)
            dtb = cp.tile([128, NDT], F); nc.sync.dma_start(dtb[:], dtb_d)
            acols = cp.tile([128, NDT * N], F); nc.sync.dma_start(acols[:], acols_d)
            dcol = cp.tile([128, NDT], F); nc.sync.dma_start(dcol[:], dcol_d)
            mb = cp.tile([128, NCT], F); nc.sync.dma_start(mb[:], mb_d)
            gb = cp.tile([128, D_GEOM // 128], F); nc.sync.dma_start(gb[:], gb_d)
            xpT = cp.tile([128, NDT * 64], FR)
            for j in range(NDT):
                nc.sync.dma_start(xpT[:, j * 64:(j + 1) * 64], xpT_d[j * 128:(j + 1) * 128, :])
            dtT = cp.tile([DT_RANK, DSH], FR); nc.sync.dma_start(dtT[:], dtT_d)
            woutT = cp.tile([128, NDT * DIM], FR)
            for j in range(NDT):
                nc.sync.dma_start(woutT[:, j * DIM:(j + 1) * DIM], woutT_d[j * 128:(j + 1) * 128, :])
            states = cp.tile([128, NDT * N], F)

            # ------- DRAM scratch -------
            xm_dram = dp.tile([DSH, L + 4], F)
            z_dram = dp.tile([DSH, L], F)
            u_dram = dp.tile([DSH, L], F)
            xdblp_dram = dp.tile([NCH * 64, TCH], F)
            xdbl_dram = dp.tile([NCH * 64, TCH], F)
            hmp_dram = dp.tile([NCH * DIM, TCH], F)
            hm_dram = dp.tile([NCH * DIM, TCH], F)

            zpad = cp.tile([128, 4], F)
            nc.vector.memset(zpad[:], 0.0)
            epsc = cp.tile([128, 1], F)
            nc.vector.memset(epsc[:], EPS)
            for j in range(NDT):
                nc.sync.dma_start(xm_dram[j * 128:(j + 1) * 128, 0:4], zpad[:])

            # ======= phase A: LN(x)->hT, in_proj, conv, silu, x_proj =======
            with tc.tile_pool(name="hp1", bufs=1) as hp1, \
                 tc.tile_pool(name="hp2", bufs=2) as hp2, \
                 tc.tile_pool(name="ppA", bufs=2, space="PSUM") as ppA:
                hT = [hp1.tile([128, L], FR, tag=f"hT{ct}", name=f"hT{ct}") for ct in range(NCT)]
                winT = []
                for kt in range(NCT):
                    wt_ = hp1.tile([128, 2 * DSH], FR, tag=f"winT{kt}", name=f"winT{kt}")
                    nc.sync.dma_start(wt_[:], winT_d[kt * 128:(kt + 1) * 128, :])
                    winT.append(wt_)
                for tt in range(NT):
                    xt = hp2.tile([128, DIM], F, tag="xt")
                    nc.sync.dma_start(xt[:], x_d[tt * 128:(tt + 1) * 128, :])
                    sm = hp2.tile([128, 1], F, tag="sm")
                    nc.vector.reduce_sum(sm[:], xt[:], axis=AX.X)
                    mu = hp2.tile([128, 1], F, tag="mu")
                    nc.scalar.mul(mu[:], sm[:], 1.0 / DIM)
                    cen = hp2.tile([128, DIM], FR, tag="cen")
                    nc.vector.tensor_scalar_sub(cen[:], xt[:], mu[:])
                    sq = hp2.tile([128, DIM], F, tag="sq")
                    vs = hp2.tile([128, 1], F, tag="vs")
                    nc.scalar.activation(sq[:], cen[:], AF.Square, accum_out=vs[:])
                    sd = hp2.tile([128, 1], F, tag="sd")
                    nc.scalar.activation(sd[:], vs[:], AF.Sqrt, bias=epsc[:], scale=1.0 / DIM)
                    rstd = hp2.tile([128, 1], F, tag="rstd")
                    nc.vector.reciprocal(rstd[:], sd[:])
                    Dg = hp2.tile([128, 128], FR, tag="Dg")
                    nc.vector.tensor_scalar_mul(Dg[:], I128[:], rstd[:])
                    for ct in range(NCT):
                        ph = ppA.tile([128, 128], F, tag="ph")
                        nc.tensor.matmul(ph[:], cen[:, ct * 128:(ct + 1) * 128], Dg[:],
                                         start=True, stop=True)
                        nc.vector.scalar_tensor_tensor(
                            hT[ct][:, tt * 128:(tt + 1) * 128], ph[:],
                            nw[:, ct:ct + 1],
                            nb[:, ct:ct + 1].broadcast_to([128, 128]),
                            ALU.mult, ALU.add)

                for ch in range(NCH):
                    c0 = ch * TCH
                    for half in range(2):          # 0: xm, 1: z
                        for mt in range(NDT):
                            px = ppA.tile([128, TCH], F, tag="px")
                            for kt in range(NCT):
                                nc.tensor.matmul(
                                    px[:],
                                    winT[kt][:, half * DSH + mt * 128: half * DSH + (mt + 1) * 128],
                                    hT[kt][:, c0:c0 + TCH],
                                    start=(kt == 0), stop=(kt == NCT - 1))
                            if half == 0:
                                xms = hp2.tile([128, TCH], F, tag="xms")
                                nc.scalar.copy(xms[:], px[:])
                                nc.sync.dma_start(
                                    xm_dram[mt * 128:(mt + 1) * 128, 4 + c0:4 + c0 + TCH], xms[:])
                            else:
                                zs = hp2.tile([128, TCH], F, tag="zs")
                                nc.scalar.activation(zs[:], px[:], AF.Silu)
                                nc.sync.dma_start(
                                    z_dram[mt * 128:(mt + 1) * 128, c0:c0 + TCH], zs[:])
                    pxp = ppA.tile([64, TCH], F, tag="pxp")
                    for j in range(NDT):
                        xmc = hp2.tile([128, TCH + 4], F, tag="xmc")
                        nc.sync.dma_start(xmc[:], xm_dram[j * 128:(j + 1) * 128, c0:c0 + TCH + 4])
                        acc = hp2.tile([128, TCH], F, tag="acc")
                        nc.vector.scalar_tensor_tensor(
                            acc[:], xmc[:, 1:1 + TCH], convw[:, j * 4:j * 4 + 1],
                            convb[:, j:j + 1].broadcast_to([128, TCH]),
                            ALU.mult, ALU.add)
                        acc2 = hp2.tile([128, TCH], F, tag="acc2")
                        nc.vector.scalar_tensor_tensor(
                            acc2[:], xmc[:, 2:2 + TCH], convw[:, j * 4 + 1:j * 4 + 2],
                            acc[:], ALU.mult, ALU.add)
                        nc.vector.scalar_tensor_tensor(
                            acc[:], xmc[:, 3:3 + TCH], convw[:, j * 4 + 2:j * 4 + 3],
                            acc2[:], ALU.mult, ALU.add)
                        nc.vector.scalar_tensor_tensor(
                            acc2[:], xmc[:, 4:4 + TCH], convw[:, j * 4 + 3:j * 4 + 4],
                            acc[:], ALU.mult, ALU.add)
                        uc = hp2.tile([128, TCH], F, tag="uc")
                        nc.scalar.activation(uc[:], acc2[:], AF.Silu)
                        nc.sync.dma_start(u_dram[j * 128:(j + 1) * 128, c0:c0 + TCH], uc[:])
                        ucr = hp2.tile([128, TCH], FR, tag="ucr")
                        nc.vector.tensor_copy(ucr[:], uc[:])
                        nc.tensor.matmul(pxp[:], xpT[:, j * 64:(j + 1) * 64], ucr[:],
                                         start=(j == 0), stop=(j == NDT - 1))
                    xps = hp2.tile([64, TCH], F, tag="xps")
                    nc.scalar.copy(xps[:], pxp[:])
                    nc.sync.dma_start(xdblp_dram[ch * 64:(ch + 1) * 64, :], xps[:])
                    if sim1:
                        nc.sync.dma_start(xdbl_dram[ch * 64:(ch + 1) * 64, :],
                                          xdblp_dram[ch * 64:(ch + 1) * 64, :])
                    else:
                        nc.gpsimd.collective_compute(
                            "AllReduce", ALU.add,
                            replica_groups=[[0, 1], [2, 3], [4, 5], [6, 7]],
                            ins=[xdblp_dram[ch * 64:(ch + 1) * 64, :]],
                            outs=[xdbl_dram[ch * 64:(ch + 1) * 64, :]])

            # ------- AllReduce x_dbl over the d_inner pair -------
            # ======= phase B: dt_proj, scan, gate, out_proj partial =======
            with tc.tile_pool(name="sp1", bufs=1) as sp1, \
                 tc.tile_pool(name="sp2", bufs=2) as sp2, \
                 tc.tile_pool(name="ppB", bufs=2, space="PSUM") as ppB:
                for ch in range(NCH):
                    c0 = ch * TCH
                    r0 = ch * 64
                    dtTf = sp2.tile([DT_RANK, TCH], F, tag="dtTf")
                    nc.sync.dma_start(dtTf[:], xdbl_dram[r0:r0 + DT_RANK, :])
                    dtTr = sp2.tile([DT_RANK, TCH], FR, tag="dtTr")
                    nc.scalar.copy(dtTr[:], dtTf[:])
                    brep = sp1.tile([128, N, TCH], F, tag="brep")
                    nc.sync.dma_start(
                        brep[:],
                        xdbl_dram[r0 + DT_RANK:r0 + DT_RANK + N, :].partition_broadcast(128))
                    crep = sp1.tile([128, N, TCH], F, tag="crep")
                    nc.sync.dma_start(
                        crep[:],
                        xdbl_dram[r0 + DT_RANK + N:r0 + DT_RANK + 2 * N, :].partition_broadcast(128))
                    y2 = []
                    for j in range(NDT):
                        pd = ppB.tile([128, TCH], F, tag="pd")
                        nc.tensor.matmul(pd[:], dtT[:, j * 128:(j + 1) * 128],
                                         dtTr[:], start=True, stop=True)
                        expd = sp2.tile([128, TCH], F, tag="expd")
                        nc.scalar.activation(expd[:], pd[:], AF.Exp, bias=dtb[:, j:j + 1])
                        delta = sp2.tile([128, TCH], F, tag="delta")
                        nc.scalar.activation(delta[:], expd[:], AF.Ln, bias=1.0)
                        uc2 = sp2.tile([128, TCH], F, tag="uc2")
                        nc.sync.dma_start(uc2[:], u_dram[j * 128:(j + 1) * 128, c0:c0 + TCH])
                        du = sp2.tile([128, TCH], F, tag="du")
                        nc.gpsimd.tensor_mul(du[:], delta[:], uc2[:])
                        ht = sp1.tile([128, N * TCH], F, tag="ht")
                        ht_nt = ht[:].rearrange("p (t n) -> p n t", n=N)   # n minor in memory
                        for n in range(N):
                            dA = sp2.tile([128, TCH], F, tag="dA", bufs=3)
                            nc.scalar.activation(dA[:], delta[:], AF.Exp,
                                                 scale=acols[:, j * N + n:j * N + n + 1])
                            dBu = sp2.tile([128, TCH], F, tag=f"dBu{n % 2}", bufs=3)
                            eng = nc.vector if n % 8 < 3 else nc.gpsimd
                            eng.tensor_mul(dBu[:], du[:], brep[:, n, :])
                            init = 0.0 if ch == 0 else states[:, j * N + n:j * N + n + 1]
                            nc.vector.tensor_tensor_scan(
                                ht_nt[:, n], dA[:], dBu[:], init,
                                ALU.mult, ALU.add)
                            if ch < NCH - 1:
                                nc.vector.tensor_copy(
                                    states[:, j * N + n:j * N + n + 1],
                                    ht[:, (TCH - 1) * N + n:(TCH - 1) * N + n + 1])
                        nh = 6
                        nc.vector.tensor_mul(ht_nt[:, :nh], ht_nt[:, :nh], crep[:, :nh])
                        nc.gpsimd.tensor_mul(ht_nt[:, nh:], ht_nt[:, nh:], crep[:, nh:])
                        yv = sp2.tile([128, TCH], F, tag="yv")
                        nc.vector.tensor_reduce(yv[:], ht[:].rearrange("p (t n) -> p t n", n=N),
                                                AX.X, ALU.add)
                        ys = sp2.tile([128, TCH], F, tag="ys")
                        nc.vector.scalar_tensor_tensor(ys[:], uc2[:], dcol[:, j:j + 1], yv[:],
                                                       ALU.mult, ALU.add)
                        zc = sp2.tile([128, TCH], F, tag="zc")
                        nc.sync.dma_start(zc[:], z_dram[j * 128:(j + 1) * 128, c0:c0 + TCH])
                        y2j = sp2.tile([128, TCH], FR, tag=f"y2_{j}", name=f"y2_{j}")
                        nc.vector.tensor_mul(y2j[:], ys[:], zc[:])
                        y2.append(y2j)
                    for mt in range(NCT):
                        po = ppB.tile([128, TCH], F, tag="po")
                        for j in range(NDT):
                            nc.tensor.matmul(
                                po[:], woutT[:, j * DIM + mt * 128: j * DIM + (mt + 1) * 128],
                                y2[j][:], start=(j == 0), stop=(j == NDT - 1))
                        hms = sp2.tile([128, TCH], F, tag="hms")
                        nc.scalar.copy(hms[:], po[:])
                        nc.sync.dma_start(
                            hmp_dram[ch * DIM + mt * 128: ch * DIM + (mt + 1) * 128, :], hms[:])
                    if sim1:
                        nc.sync.dma_start(hm_dram[ch * DIM:(ch + 1) * DIM, :],
                                          hmp_dram[ch * DIM:(ch + 1) * DIM, :])
                    else:
                        nc.gpsimd.collective_compute(
                            "AllReduce", ALU.add,
                            replica_groups=[[0, 1], [2, 3], [4, 5], [6, 7]],
                            ins=[hmp_dram[ch * DIM:(ch + 1) * DIM, :]],
                            outs=[hm_dram[ch * DIM:(ch + 1) * DIM, :]])

            # ======= phase C: match & geom heads + final LNs =======
            with tc.tile_pool(name="mp1", bufs=1) as mp1, \
                 tc.tile_pool(name="mp2", bufs=2) as mp2, \
                 tc.tile_pool(name="ppC", bufs=2, space="PSUM") as ppC:
                wmT = mp1.tile([128, NCT * DIM], FR)
                for j in range(NCT):
                    nc.sync.dma_start(wmT[:, j * DIM:(j + 1) * DIM], wmT_d[j * 128:(j + 1) * 128, :])
                wgT = mp1.tile([128, NCT * D_GEOM], FR)
                for j in range(NCT):
                    nc.sync.dma_start(wgT[:, j * D_GEOM:(j + 1) * D_GEOM], wgT_d[j * 128:(j + 1) * 128, :])
                nmw = mp1.tile([128, DIM], F); nc.sync.dma_start(nmw[:], nmw_d.broadcast_to([128, DIM]))
                nmb = mp1.tile([128, DIM], F); nc.sync.dma_start(nmb[:], nmb_d.broadcast_to([128, DIM]))
                ngw = mp1.tile([128, D_GEOM], F); nc.sync.dma_start(ngw[:], ngw_d.broadcast_to([128, D_GEOM]))
                ngb = mp1.tile([128, D_GEOM], F); nc.sync.dma_start(ngb[:], ngb_d.broadcast_to([128, D_GEOM]))

                def layer_norm_store(src, width, w_rep, b_rep, out_ap):
                    sm2 = mp2.tile([128, 1], F, tag="sm2")
                    nc.vector.reduce_sum(sm2[:], src[:], axis=AX.X)
                    mu2 = mp2.tile([128, 1], F, tag="mu2")
                    nc.scalar.mul(mu2[:], sm2[:], 1.0 / width)
                    cen2 = mp2.tile([128, width], F, tag=f"cen2_{width}")
                    nc.vector.tensor_scalar_sub(cen2[:], src[:], mu2[:])
                    sq2 = mp2.tile([128, width], F, tag=f"sq2_{width}")
                    vs2 = mp2.tile([128, 1], F, tag="vs2")
                    nc.scalar.activation(sq2[:], cen2[:], AF.Square, accum_out=vs2[:])
                    sd2 = mp2.tile([128, 1], F, tag="sd2")
                    nc.scalar.activation(sd2[:], vs2[:], AF.Sqrt, bias=epsc[:], scale=1.0 / width)
                    rstd2 = mp2.tile([128, 1], F, tag="rstd2")
                    nc.vector.reciprocal(rstd2[:], sd2[:])
                    o1 = mp2.tile([128, width], F, tag=f"o1_{width}")
                    nc.vector.scalar_tensor_tensor(o1[:], cen2[:], rstd2[:], w_rep[:, :width],
                                                   ALU.mult, ALU.mult)
                    o2 = mp2.tile([128, width], F, tag=f"o2_{width}")
                    nc.vector.tensor_add(o2[:], o1[:], b_rep[:, :width])
                    nc.sync.dma_start(out_ap, o2[:])

                for tb in range(L // TCH):
                    b0 = tb * TCH
                    hmT = []
                    for ct in range(NCT):
                        tf = mp2.tile([128, TCH], F, tag="hmTf")
                        nc.sync.dma_start(tf[:], hm_dram[tb * DIM + ct * 128: tb * DIM + (ct + 1) * 128, :])
                        t_ = mp2.tile([128, TCH], FR, tag=f"hmT{ct}", name=f"hmT{ct}")
                        nc.scalar.copy(t_[:], tf[:])
                        hmT.append(t_)
                    pms = []
                    for mt in range(NCT):
                        pm = ppC.tile([128, TCH], F, tag=f"pm{mt}", name=f"pm{mt}", bufs=1)
                        for kt in range(NCT):
                            nc.tensor.matmul(
                                pm[:], wmT[:, kt * DIM + mt * 128: kt * DIM + mt * 128 + 128],
                                hmT[kt][:],
                                start=(kt == 0), stop=(kt == NCT - 1))
                        pms.append(pm)
                    pgs = []
                    for mt in range(D_GEOM // 128):
                        pg = ppC.tile([128, TCH], F, tag=f"pg{mt}", name=f"pg{mt}", bufs=1)
                        for kt in range(NCT):
                            nc.tensor.matmul(
                                pg[:], wgT[:, kt * D_GEOM + mt * 128: kt * D_GEOM + mt * 128 + 128],
                                hmT[kt][:],
                                start=(kt == 0), stop=(kt == NCT - 1))
                        pgs.append(pg)
                    for st in range(TCH // 128):
                        t0 = b0 + st * 128
                        s0 = st * 128
                        mrow = mp2.tile([128, DIM], F, tag="mrow")
                        xres = mp2.tile([128, DIM], F, tag="xres")
                        nc.sync.dma_start(xres[:], x_d[t0:t0 + 128, :])
                        for mt in range(NCT):
                            mc = mp2.tile([128, 128], FR, tag="mc")
                            nc.vector.tensor_scalar_add(mc[:], pms[mt][:, s0:s0 + 128],
                                                        mb[:, mt:mt + 1])
                            pt = ppC.tile([128, 128], F, tag="pt", bufs=1)
                            nc.tensor.matmul(pt[:], mc[:], I128[:], start=True, stop=True)
                            nc.vector.tensor_add(mrow[:, mt * 128:(mt + 1) * 128], pt[:],
                                                 xres[:, mt * 128:(mt + 1) * 128])
                        layer_norm_store(mrow, DIM, nmw, nmb, ym_d[t0:t0 + 128, :])

                        grow = mp2.tile([128, D_GEOM], F, tag="grow")
                        for mt in range(D_GEOM // 128):
                            gc = mp2.tile([128, 128], FR, tag="gc")
                            nc.vector.tensor_scalar_add(gc[:], pgs[mt][:, s0:s0 + 128],
                                                        gb[:, mt:mt + 1])
                            pgt = ppC.tile([128, 128], F, tag="pgt", bufs=1)
                            nc.tensor.matmul(pgt[:], gc[:], I128[:], start=True, stop=True)
                            nc.vector.tensor_copy(grow[:, mt * 128:(mt + 1) * 128], pgt[:])
                        layer_norm_store(grow, D_GEOM, ngw, ngb, yg_d[t0:t0 + 128, :])

    nc.compile()
    return nc


def _host_inputs(inputs, core):
    b, s = core // 2, core % 2
    f32 = np.float32
    A = (-np.exp(np.asarray(inputs["A_log"], f32)))[s * DSH:(s + 1) * DSH]  # [512,16]
    in_proj_w = np.asarray(inputs["in_proj_w"], f32)
    winT = np.concatenate([in_proj_w[s * DSH:(s + 1) * DSH],
                           in_proj_w[D_INNER + s * DSH:D_INNER + (s + 1) * DSH]], 0).T
    conv_w = np.asarray(inputs["conv_w"], f32)[s * DSH:(s + 1) * DSH, 0, :]     # [512,4]
    conv_b = np.asarray(inputs["conv_b"], f32)[s * DSH:(s + 1) * DSH]
    xp = np.asarray(inputs["x_proj_w"], f32)[:, s * DSH:(s + 1) * DSH]          # [64, 512]
    dt_w = np.asarray(inputs["dt_proj_w"], f32)[s * DSH:(s + 1) * DSH]          # [512, 32]
    dt_b = np.asarray(inputs["dt_proj_b"], f32)[s * DSH:(s + 1) * DSH]
    Dv = np.asarray(inputs["D"], f32)[s * DSH:(s + 1) * DSH]
    wout = np.asarray(inputs["mix_out_w"], f32)[:, s * DSH:(s + 1) * DSH]       # [512, 512]

    def col128(v):        # [512] -> [128, k] (col j = slice for tile j)
        return np.ascontiguousarray(v.reshape(-1, 128).T)

    def grid128(m):       # [nt*128, k] -> [128, nt*k]
        nt = m.shape[0] // 128
        return np.ascontiguousarray(m.reshape(nt, 128, -1).transpose(1, 0, 2).reshape(128, -1))

    return {
        "x": np.ascontiguousarray(np.asarray(inputs["x"], f32)[b]),
        "i128": np.eye(128, dtype=f32),
        "winT": np.ascontiguousarray(winT),
        "nw": col128(np.asarray(inputs["norm_w"], f32)),
        "nb": col128(np.asarray(inputs["norm_b"], f32)),
        "convw": grid128(conv_w),
        "convb": col128(conv_b),
        "xpT": np.ascontiguousarray(xp.T),
        "dtT": np.ascontiguousarray(dt_w.T),
        "dtb": col128(dt_b),
        "acols": grid128(A),
        "dcol": col128(Dv),
        "woutT": np.ascontiguousarray(wout.T),
        "wmT": np.ascontiguousarray(np.asarray(inputs["match_w"], f32).T),
        "mb": col128(np.asarray(inputs["match_b"], f32)),
        "wgT": np.ascontiguousarray(np.asarray(inputs["geom_w"], f32).T),
        "gb": col128(np.asarray(inputs["geom_b"], f32)),
        "nmw": np.asarray(inputs["normm_w"], f32).reshape(1, -1),
        "nmb": np.asarray(inputs["normm_b"], f32).reshape(1, -1),
        "ngw": np.asarray(inputs["normg_w"], f32).reshape(1, -1),
        "ngb": np.asarray(inputs["normg_b"], f32).reshape(1, -1),
    }


def kernel(**inputs):
    from concourse import bass_utils
    if "nc" not in _cache:
        _cache["nc"] = _build()
    nc = _cache["nc"]
    in_maps = [_host_inputs(inputs, c) for c in range(8)]
    res = bass_utils.run_bass_kernel_spmd(nc, in_maps, core_ids=list(range(8)),
                                          trace=bool(os.environ.get("BASS_TRACE")))
    _cache["last"] = res
    if res.exec_time_ns is not None:
        print(f"HW exec time: {res.exec_time_ns} ns")
        if res.instructions_and_trace:
            print("trace:", res.instructions_and_trace[1])
    ym = np.zeros((B, L, DIM), np.float32)
    yg = np.zeros((B, L, D_GEOM), np.float32)
    for b in range(B):
        ym[b] = res.results[2 * b]["ym"]
        yg[b] = res.results[2 * b]["yg"]
    return ym, yg


# revision 29
# speedup vs baseline: 1.0350x; 1.0149x over previous
"""Trainium2 Bass kernel for nn_MultiHeadMambaBlock_79130477461819.

Sharding: 8 cores = 4 batches x 2 halves of d_inner (tensor parallel over
scan channels). x_proj / out_proj contract over full d_inner -> paired
AllReduce. Selective scan uses the DVE linear-recurrence instruction
(tensor_tensor_scan: state = dA*state + dBu along free dim), 128
channels/partitions per instruction, one scan per (d-tile, state index n).
"""
import sys, os
sys.path.insert(0, "/opt/trn_rl_repo")
os.environ.setdefault("JAX_PLATFORMS", "axon,cpu")

import numpy as np

DIM = 512
D_INNER = 1024
DSH = 512          # d_inner shard per core
N = 16             # d_state
D_CONV = 4
DT_RANK = 32
D_GEOM = 256
B, L = 4, 2048
EPS = 1e-5
TCH = 512
NCH = L // TCH     # 4 chunks
NT = L // 128      # 16 t-tiles
NDT = DSH // 128   # 4 d-tiles
NCT = DIM // 128   # 4 c-tiles
MM_FR = True       # float32r fast matmuls
SIM1 = False       # build without collectives for TimelineSim

_cache = {}


def _build(sim1=False):
    import concourse.mybir as mybir
    import concourse.bacc as bacc
    import concourse.tile as tile

    F = mybir.dt.float32
    FR = mybir.dt.float32r if MM_FR else F
    AF = mybir.ActivationFunctionType
    ALU = mybir.AluOpType
    AX = mybir.AxisListType

    nc = bacc.Bacc("TRN2", target_bir_lowering=False, debug=False,
                   num_devices=1 if sim1 else 8)

    def din(name, shape, dt=F):
        return nc.dram_tensor(name, list(shape), dt, kind="ExternalInput").ap()

    x_d = din("x", [L, DIM])                      # this core's batch, t-major
    i128_d = din("i128", [128, 128], FR)
    winT_d = din("winT", [DIM, 2 * DSH], FR)      # [c, xm|z]
    nw_d = din("nw", [128, NCT]); nb_d = din("nb", [128, NCT])
    convw_d = din("convw", [128, NDT * D_CONV]); convb_d = din("convb", [128, NDT])
    xpT_d = din("xpT", [DSH, 64], FR)
    dtT_d = din("dtT", [DT_RANK, DSH], FR)
    dtb_d = din("dtb", [128, NDT])
    acols_d = din("acols", [128, NDT * N])        # A = -exp(A_log)
    dcol_d = din("dcol", [128, NDT])              # D vec
    woutT_d = din("woutT", [DSH, DIM], FR)
    wmT_d = din("wmT", [DIM, DIM], FR)
    mb_d = din("mb", [128, NCT])
    wgT_d = din("wgT", [DIM, D_GEOM], FR)
    gb_d = din("gb", [128, D_GEOM // 128])
    nmw_d = din("nmw", [1, DIM]); nmb_d = din("nmb", [1, DIM])
    ngw_d = din("ngw", [1, D_GEOM]); ngb_d = din("ngb", [1, D_GEOM])
    ym_d = nc.dram_tensor("ym", [L, DIM], F, kind="ExternalOutput").ap()
    yg_d = nc.dram_tensor("yg", [L, D_GEOM], F, kind="ExternalOutput").ap()

    with tile.TileContext(nc) as tc:
        with tc.tile_pool(name="const", bufs=1) as cp, \
             tc.tile_pool(name="dram", bufs=1, space="DRAM") as dp:

            # ------- constants -------
            I128 = cp.tile([128, 128], FR); nc.sync.dma_start(I128[:], i128_d)
            nw = cp.tile([128, NCT], F); nc.sync.dma_start(nw[:], nw_d)
            nb = cp.tile([128, NCT], F); nc.sync.dma_start(nb[:], nb_d)
            convw = cp.tile([128, NDT * D_CONV], F); nc.sync.dma_start(convw[:], convw_d)
            convb = cp.tile([128, NDT], F); nc.sync.dma_start(convb[:], convb_d)
            dtb = cp.tile([128, NDT], F); nc.sync.dma_start(dtb[:], dtb_d)
            acols = cp.tile([128, NDT * N], F); nc.sync.dma_start(acols[:], acols_d)
            dcol = cp.tile([128, NDT], F); nc.sync.dma_start(dcol[:], dcol_d)
            mb = cp.tile([128, NCT], F); nc.sync.dma_start(mb[:], mb_d)
            gb = cp.tile([128, D_GEOM // 128], F); nc.sync.dma_start(gb[:], gb_d)
            xpT = cp.tile([128, NDT * 64], FR)
            for j in range(NDT):
                nc.sync.dma_start(xpT[:, j * 64:(j + 1) * 64], xpT_d[j * 128:(j + 1) * 128, :])
            dtT = cp.tile([DT_RANK, DSH], FR); nc.sync.dma_start(dtT[:], dtT_d)
            woutT = cp.tile([128, NDT * DIM], FR)
            for j in range(NDT):
                nc.sync.dma_start(woutT[:, j * DIM:(j + 1) * DIM], woutT_d[j * 128:(j + 1) * 128, :])
            states = cp.tile([128, NDT * N], F)

            # ------- DRAM scratch -------
            xm_dram = dp.tile([DSH, L + 4], F)
            z_dram = dp.tile([DSH, L], F)
            u_dram = dp.tile([DSH, L], F)
            xdblp_dram = dp.tile([NCH * 64, TCH], F)
            xdbl_dram = dp.tile([NCH * 64, TCH], F)
            hmp_dram = dp.tile([NCH * DIM, TCH], F)
            hm_dram = dp.tile([NCH * DIM, TCH], F)

            zpad = cp.tile([128, 4], F)
            nc.vector.memset(zpad[:], 0.0)
            epsc = cp.tile([128, 1], F)
            nc.vector.memset(epsc[:], EPS)
            for j in range(NDT):
                nc.sync.dma_start(xm_dram[j * 128:(j + 1) * 128, 0:4], zpad[:])

            # ======= phase A: LN(x)->hT, in_proj, conv, silu, x_proj =======
            with tc.tile_pool(name="hp1", bufs=1) as hp1, \
                 tc.tile_pool(name="hp2", bufs=2) as hp2, \
                 tc.tile_pool(name="ppA", bufs=2, space="PSUM") as ppA:
                hT = [hp1.tile([128, L], FR, tag=f"hT{ct}", name=f"hT{ct}") for ct in range(NCT)]
                winT = []
                for kt in range(NCT):
                    wt_ = hp1.tile([128, 2 * DSH], FR, tag=f"winT{kt}", name=f"winT{kt}")
                    nc.sync.dma_start(wt_[:], winT_d[kt * 128:(kt + 1) * 128, :])
                    winT.append(wt_)
                for tt in range(NT):
                    xt = hp2.tile([128, DIM], F, tag="xt")
                    nc.sync.dma_start(xt[:], x_d[tt * 128:(tt + 1) * 128, :])
                    sm = hp2.tile([128, 1], F, tag="sm")
                    nc.vector.reduce_sum(sm[:], xt[:], axis=AX.X)
                    mu = hp2.tile([128, 1], F, tag="mu")
                    nc.scalar.mul(mu[:], sm[:], 1.0 / DIM)
                    cen = hp2.tile([128, DIM], FR, tag="cen")
                    nc.vector.tensor_scalar_sub(cen[:], xt[:], mu[:])
                    sq = hp2.tile([128, DIM], F, tag="sq")
                    vs = hp2.tile([128, 1], F, tag="vs")
                    nc.scalar.activation(sq[:], cen[:], AF.Square, accum_out=vs[:])
                    sd = hp2.tile([128, 1], F, tag="sd")
                    nc.scalar.activation(sd[:], vs[:], AF.Sqrt, bias=epsc[:], scale=1.0 / DIM)
                    rstd = hp2.tile([128, 1], F, tag="rstd")
                    nc.vector.reciprocal(rstd[:], sd[:])
                    Dg = hp2.tile([128, 128], FR, tag="Dg")
                    nc.vector.tensor_scalar_mul(Dg[:], I128[:], rstd[:])
                    for ct in range(NCT):
                        ph = ppA.tile([128, 128], F, tag="ph")
                        nc.tensor.matmul(ph[:], cen[:, ct * 128:(ct + 1) * 128], Dg[:],
                                         start=True, stop=True)
                        nc.vector.scalar_tensor_tensor(
                            hT[ct][:, tt * 128:(tt + 1) * 128], ph[:],
                            nw[:, ct:ct + 1],
                            nb[:, ct:ct + 1].broadcast_to([128, 128]),
                            ALU.mult, ALU.add)

                for ch in range(NCH):
                    c0 = ch * TCH
                    for half in range(2):          # 0: xm, 1: z
                        for mt in range(NDT):
                            px = ppA.tile([128, TCH], F, tag="px")
                            for kt in range(NCT):
                                nc.tensor.matmul(
                                    px[:],
                                    winT[kt][:, half * DSH + mt * 128: half * DSH + (mt + 1) * 128],
                                    hT[kt][:, c0:c0 + TCH],
                                    start=(kt == 0), stop=(kt == NCT - 1))
                            if half == 0:
                                xms = hp2.tile([128, TCH], F, tag="xms")
                                nc.scalar.copy(xms[:], px[:])
                                nc.sync.dma_start(
                                    xm_dram[mt * 128:(mt + 1) * 128, 4 + c0:4 + c0 + TCH], xms[:])
                            else:
                                zs = hp2.tile([128, TCH], F, tag="zs")
                                nc.scalar.activation(zs[:], px[:], AF.Silu)
                                nc.sync.dma_start(
                                    z_dram[mt * 128:(mt + 1) * 128, c0:c0 + TCH], zs[:])
                    pxp = ppA.tile([64, TCH], F, tag="pxp")
                    for j in range(NDT):
                        xmc = hp2.tile([128, TCH + 4], F, tag="xmc")
                        nc.sync.dma_start(xmc[:], xm_dram[j * 128:(j + 1) * 128, c0:c0 + TCH + 4])
                        acc = hp2.tile([128, TCH], F, tag="acc")
                        nc.vector.scalar_tensor_tensor(
                            acc[:], xmc[:, 1:1 + TCH], convw[:, j * 4:j * 4 + 1],
                            convb[:, j:j + 1].broadcast_to([128, TCH]),
                            ALU.mult, ALU.add)
                        acc2 = hp2.tile([128, TCH], F, tag="acc2")
                        nc.vector.scalar_tensor_tensor(
                            acc2[:], xmc[:, 2:2 + TCH], convw[:, j * 4 + 1:j * 4 + 2],
                            acc[:], ALU.mult, ALU.add)
                        nc.vector.scalar_tensor_tensor(
                            acc[:], xmc[:, 3:3 + TCH], convw[:, j * 4 + 2:j * 4 + 3],
                            acc2[:], ALU.mult, ALU.add)
                        nc.vector.scalar_tensor_tensor(
                            acc2[:], xmc[:, 4:4 + TCH], convw[:, j * 4 + 3:j * 4 + 4],
                            acc[:], ALU.mult, ALU.add)
                        uc = hp2.tile([128, TCH], F, tag="uc")
                        nc.scalar.activation(uc[:], acc2[:], AF.Silu)
                        nc.sync.dma_start(u_dram[j * 128:(j + 1) * 128, c0:c0 + TCH], uc[:])
                        ucr = hp2.tile([128, TCH], FR, tag="ucr")
                        nc.vector.tensor_copy(ucr[:], uc[:])
                        nc.tensor.matmul(pxp[:], xpT[:, j * 64:(j + 1) * 64], ucr[:],
                                         start=(j == 0), stop=(j == NDT - 1))
                    xps = hp2.tile([64, TCH], F, tag="xps")
                    nc.scalar.copy(xps[:], pxp[:])
                    nc.sync.dma_start(xdblp_dram[ch * 64:(ch + 1) * 64, :], xps[:])
                    if sim1:
                        nc.sync.dma_start(xdbl_dram[ch * 64:(ch + 1) * 64, :],
                                          xdblp_dram[ch * 64:(ch + 1) * 64, :])
                    else:
                        nc.gpsimd.collective_compute(
                            "AllReduce", ALU.add,
                            replica_groups=[[0, 1], [2, 3], [4, 5], [6, 7]],
                            ins=[xdblp_dram[ch * 64:(ch + 1) * 64, :]],
                            outs=[xdbl_dram[ch * 64:(ch + 1) * 64, :]])

            # ------- AllReduce x_dbl over the d_inner pair -------
            # ======= phase B: dt_proj, scan, gate, out_proj partial =======
            with tc.tile_pool(name="sp1", bufs=1) as sp1, \
                 tc.tile_pool(name="sp2", bufs=2) as sp2, \
                 tc.tile_pool(name="ppB", bufs=2, space="PSUM") as ppB:
                for ch in range(NCH):
                    c0 = ch * TCH
                    r0 = ch * 64
                    dtTf = sp2.tile([DT_RANK, TCH], F, tag="dtTf", bufs=1)
                    nc.sync.dma_start(dtTf[:], xdbl_dram[r0:r0 + DT_RANK, :])
                    dtTr = sp2.tile([DT_RANK, TCH], FR, tag="dtTr")
                    nc.scalar.copy(dtTr[:], dtTf[:])
                    brep = sp1.tile([128, N, TCH], F, tag="brep")
                    nc.sync.dma_start(
                        brep[:],
                        xdbl_dram[r0 + DT_RANK:r0 + DT_RANK + N, :].partition_broadcast(128))
                    crep = sp1.tile([128, N, TCH], F, tag="crep")
                    nc.sync.dma_start(
                        crep[:],
                        xdbl_dram[r0 + DT_RANK + N:r0 + DT_RANK + 2 * N, :].partition_broadcast(128))
                    y2 = []
                    for j in range(NDT):
                        pd = ppB.tile([128, TCH], F, tag="pd")
                        nc.tensor.matmul(pd[:], dtT[:, j * 128:(j + 1) * 128],
                                         dtTr[:], start=True, stop=True)
                        expd = sp2.tile([128, TCH], F, tag="expd", bufs=1)
                        nc.scalar.activation(expd[:], pd[:], AF.Exp, bias=dtb[:, j:j + 1])
                        delta = sp2.tile([128, TCH], F, tag="delta")
                        nc.scalar.activation(delta[:], expd[:], AF.Ln, bias=1.0)
                        uc2 = sp2.tile([128, TCH], F, tag="uc2")
                        nc.sync.dma_start(uc2[:], u_dram[j * 128:(j + 1) * 128, c0:c0 + TCH])
                        du = sp2.tile([128, TCH], F, tag="du")
                        nc.gpsimd.tensor_mul(du[:], delta[:], uc2[:])
                        ht = sp1.tile([128, N * TCH], F, tag="ht", bufs=2)
                        ht_nt = ht[:].rearrange("p (t n) -> p n t", n=N)   # n minor in memory
                        for n in range(N):
                            dA = sp2.tile([128, TCH], F, tag="dA", bufs=3)
                            nc.scalar.activation(dA[:], delta[:], AF.Exp,
                                                 scale=acols[:, j * N + n:j * N + n + 1])
                            dBu = sp2.tile([128, TCH], F, tag=f"dBu{n % 2}", bufs=3)
                            eng = nc.vector if n % 8 < 3 else nc.gpsimd
                            eng.tensor_mul(dBu[:], du[:], brep[:, n, :])
                            init = 0.0 if ch == 0 else states[:, j * N + n:j * N + n + 1]
                            nc.vector.tensor_tensor_scan(
                                ht_nt[:, n], dA[:], dBu[:], init,
                                ALU.mult, ALU.add)
                            if ch < NCH - 1:
                                nc.vector.tensor_copy(
                                    states[:, j * N + n:j * N + n + 1],
                                    ht[:, (TCH - 1) * N + n:(TCH - 1) * N + n + 1])
                        nh = 6
                        nc.vector.tensor_mul(ht_nt[:, :nh], ht_nt[:, :nh], crep[:, :nh])
                        nc.gpsimd.tensor_mul(ht_nt[:, nh:], ht_nt[:, nh:], crep[:, nh:])
                        yv = sp2.tile([128, TCH], F, tag="yv")
                        nc.vector.tensor_reduce(yv[:], ht[:].rearrange("p (t n) -> p t n", n=N),
                                                AX.X, ALU.add)
                        ys = sp2.tile([128, TCH], F, tag="ys")
                        nc.vector.scalar_tensor_tensor(ys[:], uc2[:], dcol[:, j:j + 1], yv[:],
                                                       ALU.mult, ALU.add)
                        zc = sp2.tile([128, TCH], F, tag="zc", bufs=1)
                        nc.sync.dma_start(zc[:], z_dram[j * 128:(j + 1) * 128, c0:c0 + TCH])
                        y2j = sp2.tile([128, TCH], FR, tag=f"y2_{j}", name=f"y2_{j}")
                        nc.vector.tensor_mul(y2j[:], ys[:], zc[:])
                        y2.append(y2j)
                    for mt in range(NCT):
                        po = ppB.tile([128, TCH], F, tag="po")
                        for j in range(NDT):
                            nc.tensor.matmul(
                                po[:], woutT[:, j * DIM + mt * 128: j * DIM + (mt + 1) * 128],
                                y2[j][:], start=(j == 0), stop=(j == NDT - 1))
                        hms = sp2.tile([128, TCH], F, tag="hms", bufs=1)
                        nc.scalar.copy(hms[:], po[:])
                        nc.sync.dma_start(
                            hmp_dram[ch * DIM + mt * 128: ch * DIM + (mt + 1) * 128, :], hms[:])
                    if sim1:
                        nc.sync.dma_start(hm_dram[ch * DIM:(ch + 1) * DIM, :],
                                          hmp_dram[ch * DIM:(ch + 1) * DIM, :])
                    else:
                        nc.gpsimd.collective_compute(
                            "AllReduce", ALU.add,
                            replica_groups=[[0, 1], [2, 3], [4, 5], [6, 7]],
                            ins=[hmp_dram[ch * DIM:(ch + 1) * DIM, :]],
                            outs=[hm_dram[ch * DIM:(ch + 1) * DIM, :]])

            # ======= phase C: match & geom heads + final LNs =======
            with tc.tile_pool(name="mp1", bufs=1) as mp1, \
                 tc.tile_pool(name="mp2", bufs=2) as mp2, \
                 tc.tile_pool(name="ppC", bufs=2, space="PSUM") as ppC:
                wmT = mp1.tile([128, NCT * DIM], FR)
                for j in range(NCT):
                    nc.sync.dma_start(wmT[:, j * DIM:(j + 1) * DIM], wmT_d[j * 128:(j + 1) * 128, :])
                wgT = mp1.tile([128, NCT * D_GEOM], FR)
                for j in range(NCT):
                    nc.sync.dma_start(wgT[:, j * D_GEOM:(j + 1) * D_GEOM], wgT_d[j * 128:(j + 1) * 128, :])
                nmw = mp1.tile([128, DIM], F); nc.sync.dma_start(nmw[:], nmw_d.broadcast_to([128, DIM]))
                nmb = mp1.tile([128, DIM], F); nc.sync.dma_start(nmb[:], nmb_d.broadcast_to([128, DIM]))
                ngw = mp1.tile([128, D_GEOM], F); nc.sync.dma_start(ngw[:], ngw_d.broadcast_to([128, D_GEOM]))
                ngb = mp1.tile([128, D_GEOM], F); nc.sync.dma_start(ngb[:], ngb_d.broadcast_to([128, D_GEOM]))

                def layer_norm_store(src, width, w_rep, b_rep, out_ap):
                    sm2 = mp2.tile([128, 1], F, tag="sm2")
                    nc.vector.reduce_sum(sm2[:], src[:], axis=AX.X)
                    mu2 = mp2.tile([128, 1], F, tag="mu2")
                    nc.scalar.mul(mu2[:], sm2[:], 1.0 / width)
                    cen2 = mp2.tile([128, width], F, tag=f"cen2_{width}")
                    nc.vector.tensor_scalar_sub(cen2[:], src[:], mu2[:])
                    sq2 = mp2.tile([128, width], F, tag=f"sq2_{width}")
                    vs2 = mp2.tile([128, 1], F, tag="vs2")
                    nc.scalar.activation(sq2[:], cen2[:], AF.Square, accum_out=vs2[:])
                    sd2 = mp2.tile([128, 1], F, tag="sd2")
                    nc.scalar.activation(sd2[:], vs2[:], AF.Sqrt, bias=epsc[:], scale=1.0 / width)
                    rstd2 = mp2.tile([128, 1], F, tag="rstd2")
                    nc.vector.reciprocal(rstd2[:], sd2[:])
                    o1 = mp2.tile([128, width], F, tag=f"o1_{width}")
                    nc.vector.scalar_tensor_tensor(o1[:], cen2[:], rstd2[:], w_rep[:, :width],
                                                   ALU.mult, ALU.mult)
                    o2 = mp2.tile([128, width], F, tag=f"o2_{width}")
                    nc.vector.tensor_add(o2[:], o1[:], b_rep[:, :width])
                    nc.sync.dma_start(out_ap, o2[:])

                for tb in range(L // TCH):
                    b0 = tb * TCH
                    hmT = []
                    for ct in range(NCT):
                        tf = mp2.tile([128, TCH], F, tag="hmTf")
                        nc.sync.dma_start(tf[:], hm_dram[tb * DIM + ct * 128: tb * DIM + (ct + 1) * 128, :])
                        t_ = mp2.tile([128, TCH], FR, tag=f"hmT{ct}", name=f"hmT{ct}")
                        nc.scalar.copy(t_[:], tf[:])
                        hmT.append(t_)
                    pms = []
                    for mt in range(NCT):
                        pm = ppC.tile([128, TCH], F, tag=f"pm{mt}", name=f"pm{mt}", bufs=1)
                        for kt in range(NCT):
                            nc.tensor.matmul(
                                pm[:], wmT[:, kt * DIM + mt * 128: kt * DIM + mt * 128 + 128],
                                hmT[kt][:],
                                start=(kt == 0), stop=(kt == NCT - 1))
                        pms.append(pm)
                    pgs = []
                    for mt in range(D_GEOM // 128):
                        pg = ppC.tile([128, TCH], F, tag=f"pg{mt}", name=f"pg{mt}", bufs=1)
                        for kt in range(NCT):
                            nc.tensor.matmul(
                                pg[:], wgT[:, kt * D_GEOM + mt * 128: kt * D_GEOM + mt * 128 + 128],
                                hmT[kt][:],
                                start=(kt == 0), stop=(kt == NCT - 1))
                        pgs.append(pg)
                    for st in range(TCH // 128):
                        t0 = b0 + st * 128
                        s0 = st * 128
                        mrow = mp2.tile([128, DIM], F, tag="mrow")
                        xres = mp2.tile([128, DIM], F, tag="xres")
                        nc.sync.dma_start(xres[:], x_d[t0:t0 + 128, :])
                        for mt in range(NCT):
                            mc = mp2.tile([128, 128], FR, tag="mc")
                            nc.vector.tensor_scalar_add(mc[:], pms[mt][:, s0:s0 + 128],
                                                        mb[:, mt:mt + 1])
                            pt = ppC.tile([128, 128], F, tag="pt", bufs=1)
                            nc.tensor.matmul(pt[:], mc[:], I128[:], start=True, stop=True)
                            nc.vector.tensor_add(mrow[:, mt * 128:(mt + 1) * 128], pt[:],
                                                 xres[:, mt * 128:(mt + 1) * 128])
                        layer_norm_store(mrow, DIM, nmw, nmb, ym_d[t0:t0 + 128, :])

                        grow = mp2.tile([128, D_GEOM], F, tag="grow")
                        for mt in range(D_GEOM // 128):
                            gc = mp2.tile([128, 128], FR, tag="gc")
                            nc.vector.tensor_scalar_add(gc[:], pgs[mt][:, s0:s0 + 128],
                                                        gb[:, mt:mt + 1])
                            pgt = ppC.tile([128, 128], F, tag="pgt", bufs=1)
                            nc.tensor.matmul(pgt[:], gc[:], I128[:], start=True, stop=True)
                            nc.vector.tensor_copy(grow[:, mt * 128:(mt + 1) * 128], pgt[:])
                        layer_norm_store(grow, D_GEOM, ngw, ngb, yg_d[t0:t0 + 128, :])

    nc.compile()
    return nc


def _host_inputs(inputs, core):
    b, s = core // 2, core % 2
    f32 = np.float32
    A = (-np.exp(np.asarray(inputs["A_log"], f32)))[s * DSH:(s + 1) * DSH]  # [512,16]
    in_proj_w = np.asarray(inputs["in_proj_w"], f32)
    winT = np.concatenate([in_proj_w[s * DSH:(s + 1) * DSH],
                           in_proj_w[D_INNER + s * DSH:D_INNER + (s + 1) * DSH]], 0).T
    conv_w = np.asarray(inputs["conv_w"], f32)[s * DSH:(s + 1) * DSH, 0, :]     # [512,4]
    conv_b = np.asarray(inputs["conv_b"], f32)[s * DSH:(s + 1) * DSH]
    xp = np.asarray(inputs["x_proj_w"], f32)[:, s * DSH:(s + 1) * DSH]          # [64, 512]
    dt_w = np.asarray(inputs["dt_proj_w"], f32)[s * DSH:(s + 1) * DSH]          # [512, 32]
    dt_b = np.asarray(inputs["dt_proj_b"], f32)[s * DSH:(s + 1) * DSH]
    Dv = np.asarray(inputs["D"], f32)[s * DSH:(s + 1) * DSH]
    wout = np.asarray(inputs["mix_out_w"], f32)[:, s * DSH:(s + 1) * DSH]       # [512, 512]

    def col128(v):        # [512] -> [128, k] (col j = slice for tile j)
        return np.ascontiguousarray(v.reshape(-1, 128).T)

    def grid128(m):       # [nt*128, k] -> [128, nt*k]
        nt = m.shape[0] // 128
        return np.ascontiguousarray(m.reshape(nt, 128, -1).transpose(1, 0, 2).reshape(128, -1))

    return {
        "x": np.ascontiguousarray(np.asarray(inputs["x"], f32)[b]),
        "i128": np.eye(128, dtype=f32),
        "winT": np.ascontiguousarray(winT),
        "nw": col128(np.asarray(inputs["norm_w"], f32)),
        "nb": col128(np.asarray(inputs["norm_b"], f32)),
        "convw": grid128(conv_w),
        "convb": col128(conv_b),
        "xpT": np.ascontiguousarray(xp.T),
        "dtT": np.ascontiguousarray(dt_w.T),
        "dtb": col128(dt_b),
        "acols": grid128(A),
        "dcol": col128(Dv),
        "woutT": np.ascontiguousarray(wout.T),
        "wmT": np.ascontiguousarray(np.asarray(inputs["match_w"], f32).T),
        "mb": col128(np.asarray(inputs["match_b"], f32)),
        "wgT": np.ascontiguousarray(np.asarray(inputs["geom_w"], f32).T),
        "gb": col128(np.asarray(inputs["geom_b"], f32)),
        "nmw": np.asarray(inputs["normm_w"], f32).reshape(1, -1),
        "nmb": np.asarray(inputs["normm_b"], f32).reshape(1, -1),
        "ngw": np.asarray(inputs["normg_w"], f32).reshape(1, -1),
        "ngb": np.asarray(inputs["normg_b"], f32).reshape(1, -1),
    }


def kernel(**inputs):
    from concourse import bass_utils
    if "nc" not in _cache:
        _cache["nc"] = _build()
    nc = _cache["nc"]
    in_maps = [_host_inputs(inputs, c) for c in range(8)]
    res = bass_utils.run_bass_kernel_spmd(nc, in_maps, core_ids=list(range(8)),
                                          trace=bool(os.environ.get("BASS_TRACE")))
    _cache["last"] = res
    if res.exec_time_ns is not None:
        print(f"HW exec time: {res.exec_time_ns} ns")
        if res.instructions_and_trace:
            print("trace:", res.instructions_and_trace[1])
    ym = np.zeros((B, L, DIM), np.float32)
    yg = np.zeros((B, L, D_GEOM), np.float32)
    for b in range(B):
        ym[b] = res.results[2 * b]["ym"]
        yg[b] = res.results[2 * b]["yg"]
    return ym, yg


# revision 30
# speedup vs baseline: 1.0490x; 1.0135x over previous
"""Trainium2 Bass kernel for nn_MultiHeadMambaBlock_79130477461819.

Sharding: 8 cores = 4 batches x 2 halves of d_inner (tensor parallel over
scan channels). x_proj / out_proj contract over full d_inner -> paired
AllReduce. Selective scan uses the DVE linear-recurrence instruction
(tensor_tensor_scan: state = dA*state + dBu along free dim), 128
channels/partitions per instruction, one scan per (d-tile, state index n).
"""
import sys, os
sys.path.insert(0, "/opt/trn_rl_repo")
os.environ.setdefault("JAX_PLATFORMS", "axon,cpu")

import numpy as np

DIM = 512
D_INNER = 1024
DSH = 512          # d_inner shard per core
N = 16             # d_state
D_CONV = 4
DT_RANK = 32
D_GEOM = 256
B, L = 4, 2048
EPS = 1e-5
TCH = 512
NCH = L // TCH     # 4 chunks
NT = L // 128      # 16 t-tiles
NDT = DSH // 128   # 4 d-tiles
NCT = DIM // 128   # 4 c-tiles
MM_FR = True       # float32r fast matmuls
SIM1 = False       # build without collectives for TimelineSim

_cache = {}


def _build(sim1=False):
    import concourse.mybir as mybir
    import concourse.bacc as bacc
    import concourse.tile as tile

    F = mybir.dt.float32
    FR = mybir.dt.float32r if MM_FR else F
    AF = mybir.ActivationFunctionType
    ALU = mybir.AluOpType
    AX = mybir.AxisListType

    nc = bacc.Bacc("TRN2", target_bir_lowering=False, debug=False,
                   num_devices=1 if sim1 else 8)

    def din(name, shape, dt=F):
        return nc.dram_tensor(name, list(shape), dt, kind="ExternalInput").ap()

    x_d = din("x", [L, DIM])                      # this core's batch, t-major
    i128_d = din("i128", [128, 128], FR)
    winT_d = din("winT", [DIM, 2 * DSH], FR)      # [c, xm|z]
    nw_d = din("nw", [128, NCT]); nb_d = din("nb", [128, NCT])
    convw_d = din("convw", [128, NDT * D_CONV]); convb_d = din("convb", [128, NDT])
    xpT_d = din("xpT", [DSH, 64], FR)
    dtT_d = din("dtT", [DT_RANK, DSH], FR)
    dtb_d = din("dtb", [128, NDT])
    acols_d = din("acols", [128, NDT * N])        # A = -exp(A_log)
    dcol_d = din("dcol", [128, NDT])              # D vec
    woutT_d = din("woutT", [DSH, DIM], FR)
    wmT_d = din("wmT", [DIM, DIM], FR)
    mb_d = din("mb", [128, NCT])
    wgT_d = din("wgT", [DIM, D_GEOM], FR)
    gb_d = din("gb", [128, D_GEOM // 128])
    nmw_d = din("nmw", [1, DIM]); nmb_d = din("nmb", [1, DIM])
    ngw_d = din("ngw", [1, D_GEOM]); ngb_d = din("ngb", [1, D_GEOM])
    ym_d = nc.dram_tensor("ym", [L, DIM], F, kind="ExternalOutput").ap()
    yg_d = nc.dram_tensor("yg", [L, D_GEOM], F, kind="ExternalOutput").ap()

    with tile.TileContext(nc) as tc:
        with tc.tile_pool(name="const", bufs=1) as cp, \
             tc.tile_pool(name="dram", bufs=1, space="DRAM") as dp:

            # ------- constants -------
            I128 = cp.tile([128, 128], FR); nc.sync.dma_start(I128[:], i128_d)
            nw = cp.tile([128, NCT], F); nc.sync.dma_start(nw[:], nw_d)
            nb = cp.tile([128, NCT], F); nc.sync.dma_start(nb[:], nb_d)
            convw = cp.tile([128, NDT * D_CONV], F); nc.sync.dma_start(convw[:], convw_d)
            convb = cp.tile([128, NDT], F); nc.sync.dma_start(convb[:], convb_d)
            dtb = cp.tile([128, NDT], F); nc.sync.dma_start(dtb[:], dtb_d)
            acols = cp.tile([128, NDT * N], F); nc.sync.dma_start(acols[:], acols_d)
            dcol = cp.tile([128, NDT], F); nc.sync.dma_start(dcol[:], dcol_d)
            mb = cp.tile([128, NCT], F); nc.sync.dma_start(mb[:], mb_d)
            gb = cp.tile([128, D_GEOM // 128], F); nc.sync.dma_start(gb[:], gb_d)
            xpT = cp.tile([128, NDT * 64], FR)
            for j in range(NDT):
                nc.sync.dma_start(xpT[:, j * 64:(j + 1) * 64], xpT_d[j * 128:(j + 1) * 128, :])
            dtT = cp.tile([DT_RANK, DSH], FR); nc.sync.dma_start(dtT[:], dtT_d)
            woutT = cp.tile([128, NDT * DIM], FR)
            for j in range(NDT):
                nc.sync.dma_start(woutT[:, j * DIM:(j + 1) * DIM], woutT_d[j * 128:(j + 1) * 128, :])
            states = cp.tile([128, NDT * N], F)

            # ------- DRAM scratch -------
            xm_dram = dp.tile([DSH, L + 4], F)
            z_dram = dp.tile([DSH, L], F)
            u_dram = dp.tile([DSH, L], F)
            xdblp_dram = dp.tile([NCH * 64, TCH], F)
            xdbl_dram = dp.tile([NCH * 64, TCH], F)
            hmp_dram = dp.tile([NCH * DIM, TCH], F)
            hm_dram = dp.tile([NCH * DIM, TCH], F)

            zpad = cp.tile([128, 4], F)
            nc.vector.memset(zpad[:], 0.0)
            epsc = cp.tile([128, 1], F)
            nc.vector.memset(epsc[:], EPS)
            for j in range(NDT):
                nc.sync.dma_start(xm_dram[j * 128:(j + 1) * 128, 0:4], zpad[:])

            # ======= phase A: LN(x)->hT, in_proj, conv, silu, x_proj =======
            with tc.tile_pool(name="hp1", bufs=1) as hp1, \
                 tc.tile_pool(name="hp2", bufs=2) as hp2, \
                 tc.tile_pool(name="ppA", bufs=2, space="PSUM") as ppA:
                hT = [hp1.tile([128, L], FR, tag=f"hT{ct}", name=f"hT{ct}") for ct in range(NCT)]
                winT = []
                for kt in range(NCT):
                    wt_ = hp1.tile([128, 2 * DSH], FR, tag=f"winT{kt}", name=f"winT{kt}")
                    nc.sync.dma_start(wt_[:], winT_d[kt * 128:(kt + 1) * 128, :])
                    winT.append(wt_)
                for tt in range(NT):
                    xt = hp2.tile([128, DIM], F, tag="xt")
                    nc.sync.dma_start(xt[:], x_d[tt * 128:(tt + 1) * 128, :])
                    sm = hp2.tile([128, 1], F, tag="sm")
                    nc.vector.reduce_sum(sm[:], xt[:], axis=AX.X)
                    mu = hp2.tile([128, 1], F, tag="mu")
                    nc.scalar.mul(mu[:], sm[:], 1.0 / DIM)
                    cen = hp2.tile([128, DIM], FR, tag="cen")
                    nc.vector.tensor_scalar_sub(cen[:], xt[:], mu[:])
                    sq = hp2.tile([128, DIM], F, tag="sq")
                    vs = hp2.tile([128, 1], F, tag="vs")
                    nc.scalar.activation(sq[:], cen[:], AF.Square, accum_out=vs[:])
                    sd = hp2.tile([128, 1], F, tag="sd")
                    nc.scalar.activation(sd[:], vs[:], AF.Sqrt, bias=epsc[:], scale=1.0 / DIM)
                    rstd = hp2.tile([128, 1], F, tag="rstd")
                    nc.vector.reciprocal(rstd[:], sd[:])
                    Dg = hp2.tile([128, 128], FR, tag="Dg")
                    nc.vector.tensor_scalar_mul(Dg[:], I128[:], rstd[:])
                    for ct in range(NCT):
                        ph = ppA.tile([128, 128], F, tag="ph")
                        nc.tensor.matmul(ph[:], cen[:, ct * 128:(ct + 1) * 128], Dg[:],
                                         start=True, stop=True)
                        nc.vector.scalar_tensor_tensor(
                            hT[ct][:, tt * 128:(tt + 1) * 128], ph[:],
                            nw[:, ct:ct + 1],
                            nb[:, ct:ct + 1].broadcast_to([128, 128]),
                            ALU.mult, ALU.add)

                for ch in range(NCH):
                    c0 = ch * TCH
                    for half in range(2):          # 0: xm, 1: z
                        for mt in range(NDT):
                            px = ppA.tile([128, TCH], F, tag="px")
                            for kt in range(NCT):
                                nc.tensor.matmul(
                                    px[:],
                                    winT[kt][:, half * DSH + mt * 128: half * DSH + (mt + 1) * 128],
                                    hT[kt][:, c0:c0 + TCH],
                                    start=(kt == 0), stop=(kt == NCT - 1))
                            if half == 0:
                                xms = hp2.tile([128, TCH], F, tag="xms")
                                nc.scalar.copy(xms[:], px[:])
                                nc.sync.dma_start(
                                    xm_dram[mt * 128:(mt + 1) * 128, 4 + c0:4 + c0 + TCH], xms[:])
                            else:
                                zs = hp2.tile([128, TCH], F, tag="zs")
                                nc.scalar.activation(zs[:], px[:], AF.Silu)
                                nc.sync.dma_start(
                                    z_dram[mt * 128:(mt + 1) * 128, c0:c0 + TCH], zs[:])
                    pxp = ppA.tile([64, TCH], F, tag="pxp")
                    for j in range(NDT):
                        xmc = hp2.tile([128, TCH + 4], F, tag="xmc")
                        nc.sync.dma_start(xmc[:], xm_dram[j * 128:(j + 1) * 128, c0:c0 + TCH + 4])
                        acc = hp2.tile([128, TCH], F, tag="acc")
                        nc.vector.scalar_tensor_tensor(
                            acc[:], xmc[:, 1:1 + TCH], convw[:, j * 4:j * 4 + 1],
                            convb[:, j:j + 1].broadcast_to([128, TCH]),
                            ALU.mult, ALU.add)
                        acc2 = hp2.tile([128, TCH], F, tag="acc2")
                        nc.vector.scalar_tensor_tensor(
                            acc2[:], xmc[:, 2:2 + TCH], convw[:, j * 4 + 1:j * 4 + 2],
                            acc[:], ALU.mult, ALU.add)
                        nc.vector.scalar_tensor_tensor(
                            acc[:], xmc[:, 3:3 + TCH], convw[:, j * 4 + 2:j * 4 + 3],
                            acc2[:], ALU.mult, ALU.add)
                        nc.vector.scalar_tensor_tensor(
                            acc2[:], xmc[:, 4:4 + TCH], convw[:, j * 4 + 3:j * 4 + 4],
                            acc[:], ALU.mult, ALU.add)
                        uc = hp2.tile([128, TCH], F, tag="uc")
                        nc.scalar.activation(uc[:], acc2[:], AF.Silu)
                        nc.sync.dma_start(u_dram[j * 128:(j + 1) * 128, c0:c0 + TCH], uc[:])
                        ucr = hp2.tile([128, TCH], FR, tag="ucr")
                        nc.vector.tensor_copy(ucr[:], uc[:])
                        nc.tensor.matmul(pxp[:], xpT[:, j * 64:(j + 1) * 64], ucr[:],
                                         start=(j == 0), stop=(j == NDT - 1))
                    xps = hp2.tile([64, TCH], F, tag="xps")
                    nc.scalar.copy(xps[:], pxp[:])
                    nc.sync.dma_start(xdblp_dram[ch * 64:(ch + 1) * 64, :], xps[:])
                    if sim1:
                        nc.sync.dma_start(xdbl_dram[ch * 64:(ch + 1) * 64, :],
                                          xdblp_dram[ch * 64:(ch + 1) * 64, :])
                    else:
                        nc.gpsimd.collective_compute(
                            "AllReduce", ALU.add,
                            replica_groups=[[0, 1], [2, 3], [4, 5], [6, 7]],
                            ins=[xdblp_dram[ch * 64:(ch + 1) * 64, :]],
                            outs=[xdbl_dram[ch * 64:(ch + 1) * 64, :]])

            # ------- AllReduce x_dbl over the d_inner pair -------
            # ======= phase B: dt_proj, scan, gate, out_proj partial =======
            with tc.tile_pool(name="sp1", bufs=1) as sp1, \
                 tc.tile_pool(name="sp2", bufs=2) as sp2, \
                 tc.tile_pool(name="ppB", bufs=2, space="PSUM") as ppB:
                for ch in range(NCH):
                    c0 = ch * TCH
                    r0 = ch * 64
                    dtTf = sp2.tile([DT_RANK, TCH], F, tag="dtTf", bufs=1)
                    nc.sync.dma_start(dtTf[:], xdbl_dram[r0:r0 + DT_RANK, :])
                    dtTr = sp2.tile([DT_RANK, TCH], FR, tag="dtTr")
                    nc.scalar.copy(dtTr[:], dtTf[:])
                    brep = sp1.tile([128, N, TCH], F, tag="brep")
                    nc.sync.dma_start(
                        brep[:],
                        xdbl_dram[r0 + DT_RANK:r0 + DT_RANK + N, :].partition_broadcast(128))
                    crep = sp1.tile([128, N, TCH], F, tag="crep")
                    nc.sync.dma_start(
                        crep[:],
                        xdbl_dram[r0 + DT_RANK + N:r0 + DT_RANK + 2 * N, :].partition_broadcast(128))
                    y2 = []
                    for j in range(NDT):
                        pd = ppB.tile([128, TCH], F, tag="pd")
                        nc.tensor.matmul(pd[:], dtT[:, j * 128:(j + 1) * 128],
                                         dtTr[:], start=True, stop=True)
                        expd = sp2.tile([128, TCH], F, tag="expd", bufs=1)
                        nc.scalar.activation(expd[:], pd[:], AF.Exp, bias=dtb[:, j:j + 1])
                        delta = sp2.tile([128, TCH], F, tag="delta")
                        nc.scalar.activation(delta[:], expd[:], AF.Ln, bias=1.0)
                        uc2 = sp2.tile([128, TCH], F, tag="uc2")
                        nc.sync.dma_start(uc2[:], u_dram[j * 128:(j + 1) * 128, c0:c0 + TCH])
                        du = sp2.tile([128, TCH], F, tag="du")
                        nc.gpsimd.tensor_mul(du[:], delta[:], uc2[:])
                        ht = sp1.tile([128, N * TCH], F, tag="ht", bufs=2)
                        ht_nt = ht[:].rearrange("p (t n) -> p n t", n=N)   # n minor in memory
                        for n in range(N):
                            dA = sp2.tile([128, TCH], F, tag="dA", bufs=3)
                            nc.scalar.activation(dA[:], delta[:], AF.Exp,
                                                 scale=acols[:, j * N + n:j * N + n + 1])
                            dBu = sp2.tile([128, TCH], F, tag=f"dBu{n % 2}", bufs=3)
                            eng = nc.vector if n % 8 < 2 else nc.gpsimd
                            eng.tensor_mul(dBu[:], du[:], brep[:, n, :])
                            init = 0.0 if ch == 0 else states[:, j * N + n:j * N + n + 1]
                            nc.vector.tensor_tensor_scan(
                                ht_nt[:, n], dA[:], dBu[:], init,
                                ALU.mult, ALU.add)
                            if ch < NCH - 1:
                                nc.vector.tensor_copy(
                                    states[:, j * N + n:j * N + n + 1],
                                    ht[:, (TCH - 1) * N + n:(TCH - 1) * N + n + 1])
                        nh = 6
                        nc.vector.tensor_mul(ht_nt[:, :nh], ht_nt[:, :nh], crep[:, :nh])
                        nc.gpsimd.tensor_mul(ht_nt[:, nh:], ht_nt[:, nh:], crep[:, nh:])
                        yv = sp2.tile([128, TCH], F, tag="yv")
                        nc.vector.tensor_reduce(yv[:], ht[:].rearrange("p (t n) -> p t n", n=N),
                                                AX.X, ALU.add)
                        ys = sp2.tile([128, TCH], F, tag="ys")
                        nc.vector.scalar_tensor_tensor(ys[:], uc2[:], dcol[:, j:j + 1], yv[:],
                                                       ALU.mult, ALU.add)
                        zc = sp2.tile([128, TCH], F, tag="zc", bufs=1)
                        nc.sync.dma_start(zc[:], z_dram[j * 128:(j + 1) * 128, c0:c0 + TCH])
                        y2j = sp2.tile([128, TCH], FR, tag=f"y2_{j}", name=f"y2_{j}")
                        nc.gpsimd.tensor_mul(y2j[:], ys[:], zc[:])
                        y2.append(y2j)
                    for mt in range(NCT):
                        po = ppB.tile([128, TCH], F, tag="po")
                        for j in range(NDT):
                            nc.tensor.matmul(
                                po[:], woutT[:, j * DIM + mt * 128: j * DIM + (mt + 1) * 128],
                                y2[j][:], start=(j == 0), stop=(j == NDT - 1))
                        hms = sp2.tile([128, TCH], F, tag="hms", bufs=1)
                        nc.scalar.copy(hms[:], po[:])
                        nc.sync.dma_start(
                            hmp_dram[ch * DIM + mt * 128: ch * DIM + (mt + 1) * 128, :], hms[:])
                    if sim1:
                        nc.sync.dma_start(hm_dram[ch * DIM:(ch + 1) * DIM, :],
                                          hmp_dram[ch * DIM:(ch + 1) * DIM, :])
                    else:
                        nc.gpsimd.collective_compute(
                            "AllReduce", ALU.add,
                            replica_groups=[[0, 1], [2, 3], [4, 5], [6, 7]],
                            ins=[hmp_dram[ch * DIM:(ch + 1) * DIM, :]],
                            outs=[hm_dram[ch * DIM:(ch + 1) * DIM, :]])

            # ======= phase C: match & geom heads + final LNs =======
            with tc.tile_pool(name="mp1", bufs=1) as mp1, \
                 tc.tile_pool(name="mp2", bufs=2) as mp2, \
                 tc.tile_pool(name="ppC", bufs=2, space="PSUM") as ppC:
                wmT = mp1.tile([128, NCT * DIM], FR)
                for j in range(NCT):
                    nc.sync.dma_start(wmT[:, j * DIM:(j + 1) * DIM], wmT_d[j * 128:(j + 1) * 128, :])
                wgT = mp1.tile([128, NCT * D_GEOM], FR)
                for j in range(NCT):
                    nc.sync.dma_start(wgT[:, j * D_GEOM:(j + 1) * D_GEOM], wgT_d[j * 128:(j + 1) * 128, :])
                nmw = mp1.tile([128, DIM], F); nc.sync.dma_start(nmw[:], nmw_d.broadcast_to([128, DIM]))
                nmb = mp1.tile([128, DIM], F); nc.sync.dma_start(nmb[:], nmb_d.broadcast_to([128, DIM]))
                ngw = mp1.tile([128, D_GEOM], F); nc.sync.dma_start(ngw[:], ngw_d.broadcast_to([128, D_GEOM]))
                ngb = mp1.tile([128, D_GEOM], F); nc.sync.dma_start(ngb[:], ngb_d.broadcast_to([128, D_GEOM]))

                def layer_norm_store(src, width, w_rep, b_rep, out_ap):
                    sm2 = mp2.tile([128, 1], F, tag="sm2")
                    nc.vector.reduce_sum(sm2[:], src[:], axis=AX.X)
                    mu2 = mp2.tile([128, 1], F, tag="mu2")
                    nc.scalar.mul(mu2[:], sm2[:], 1.0 / width)
                    cen2 = mp2.tile([128, width], F, tag=f"cen2_{width}")
                    nc.vector.tensor_scalar_sub(cen2[:], src[:], mu2[:])
                    sq2 = mp2.tile([128, width], F, tag=f"sq2_{width}")
                    vs2 = mp2.tile([128, 1], F, tag="vs2")
                    nc.scalar.activation(sq2[:], cen2[:], AF.Square, accum_out=vs2[:])
                    sd2 = mp2.tile([128, 1], F, tag="sd2")
                    nc.scalar.activation(sd2[:], vs2[:], AF.Sqrt, bias=epsc[:], scale=1.0 / width)
                    rstd2 = mp2.tile([128, 1], F, tag="rstd2")
                    nc.vector.reciprocal(rstd2[:], sd2[:])
                    o1 = mp2.tile([128, width], F, tag=f"o1_{width}")
                    nc.vector.scalar_tensor_tensor(o1[:], cen2[:], rstd2[:], w_rep[:, :width],
                                                   ALU.mult, ALU.mult)
                    o2 = mp2.tile([128, width], F, tag=f"o2_{width}")
                    nc.vector.tensor_add(o2[:], o1[:], b_rep[:, :width])
                    nc.sync.dma_start(out_ap, o2[:])

                for tb in range(L // TCH):
                    b0 = tb * TCH
                    hmT = []
                    for ct in range(NCT):
                        tf = mp2.tile([128, TCH], F, tag="hmTf")
                        nc.sync.dma_start(tf[:], hm_dram[tb * DIM + ct * 128: tb * DIM + (ct + 1) * 128, :])
                        t_ = mp2.tile([128, TCH], FR, tag=f"hmT{ct}", name=f"hmT{ct}")
                        nc.scalar.copy(t_[:], tf[:])
                        hmT.append(t_)
                    pms = []
                    for mt in range(NCT):
                        pm = ppC.tile([128, TCH], F, tag=f"pm{mt}", name=f"pm{mt}", bufs=1)
                        for kt in range(NCT):
                            nc.tensor.matmul(
                                pm[:], wmT[:, kt * DIM + mt * 128: kt * DIM + mt * 128 + 128],
                                hmT[kt][:],
                                start=(kt == 0), stop=(kt == NCT - 1))
                        pms.append(pm)
                    pgs = []
                    for mt in range(D_GEOM // 128):
                        pg = ppC.tile([128, TCH], F, tag=f"pg{mt}", name=f"pg{mt}", bufs=1)
                        for kt in range(NCT):
                            nc.tensor.matmul(
                                pg[:], wgT[:, kt * D_GEOM + mt * 128: kt * D_GEOM + mt * 128 + 128],
                                hmT[kt][:],
                                start=(kt == 0), stop=(kt == NCT - 1))
                        pgs.append(pg)
                    for st in range(TCH // 128):
                        t0 = b0 + st * 128
                        s0 = st * 128
                        mrow = mp2.tile([128, DIM], F, tag="mrow")
                        xres = mp2.tile([128, DIM], F, tag="xres")
                        nc.sync.dma_start(xres[:], x_d[t0:t0 + 128, :])
                        for mt in range(NCT):
                            mc = mp2.tile([128, 128], FR, tag="mc")
                            nc.vector.tensor_scalar_add(mc[:], pms[mt][:, s0:s0 + 128],
                                                        mb[:, mt:mt + 1])
                            pt = ppC.tile([128, 128], F, tag="pt", bufs=1)
                            nc.tensor.matmul(pt[:], mc[:], I128[:], start=True, stop=True)
                            nc.vector.tensor_add(mrow[:, mt * 128:(mt + 1) * 128], pt[:],
                                                 xres[:, mt * 128:(mt + 1) * 128])
                        layer_norm_store(mrow, DIM, nmw, nmb, ym_d[t0:t0 + 128, :])

                        grow = mp2.tile([128, D_GEOM], F, tag="grow")
                        for mt in range(D_GEOM // 128):
                            gc = mp2.tile([128, 128], FR, tag="gc")
                            nc.vector.tensor_scalar_add(gc[:], pgs[mt][:, s0:s0 + 128],
                                                        gb[:, mt:mt + 1])
                            pgt = ppC.tile([128, 128], F, tag="pgt", bufs=1)
                            nc.tensor.matmul(pgt[:], gc[:], I128[:], start=True, stop=True)
                            nc.vector.tensor_copy(grow[:, mt * 128:(mt + 1) * 128], pgt[:])
                        layer_norm_store(grow, D_GEOM, ngw, ngb, yg_d[t0:t0 + 128, :])

    nc.compile()
    return nc


def _host_inputs(inputs, core):
    b, s = core // 2, core % 2
    f32 = np.float32
    A = (-np.exp(np.asarray(inputs["A_log"], f32)))[s * DSH:(s + 1) * DSH]  # [512,16]
    in_proj_w = np.asarray(inputs["in_proj_w"], f32)
    winT = np.concatenate([in_proj_w[s * DSH:(s + 1) * DSH],
                           in_proj_w[D_INNER + s * DSH:D_INNER + (s + 1) * DSH]], 0).T
    conv_w = np.asarray(inputs["conv_w"], f32)[s * DSH:(s + 1) * DSH, 0, :]     # [512,4]
    conv_b = np.asarray(inputs["conv_b"], f32)[s * DSH:(s + 1) * DSH]
    xp = np.asarray(inputs["x_proj_w"], f32)[:, s * DSH:(s + 1) * DSH]          # [64, 512]
    dt_w = np.asarray(inputs["dt_proj_w"], f32)[s * DSH:(s + 1) * DSH]          # [512, 32]
    dt_b = np.asarray(inputs["dt_proj_b"], f32)[s * DSH:(s + 1) * DSH]
    Dv = np.asarray(inputs["D"], f32)[s * DSH:(s + 1) * DSH]
    wout = np.asarray(inputs["mix_out_w"], f32)[:, s * DSH:(s + 1) * DSH]       # [512, 512]

    def col128(v):        # [512] -> [128, k] (col j = slice for tile j)
        return np.ascontiguousarray(v.reshape(-1, 128).T)

    def grid128(m):       # [nt*128, k] -> [128, nt*k]
        nt = m.shape[0] // 128
        return np.ascontiguousarray(m.reshape(nt, 128, -1).transpose(1, 0, 2).reshape(128, -1))

    return {
        "x": np.ascontiguousarray(np.asarray(inputs["x"], f32)[b]),
        "i128": np.eye(128, dtype=f32),
        "winT": np.ascontiguousarray(winT),
        "nw": col128(np.asarray(inputs["norm_w"], f32)),
        "nb": col128(np.asarray(inputs["norm_b"], f32)),
        "convw": grid128(conv_w),
        "convb": col128(conv_b),
        "xpT": np.ascontiguousarray(xp.T),
        "dtT": np.ascontiguousarray(dt_w.T),
        "dtb": col128(dt_b),
        "acols": grid128(A),
        "dcol": col128(Dv),
        "woutT": np.ascontiguousarray(wout.T),
        "wmT": np.ascontiguousarray(np.asarray(inputs["match_w"], f32).T),
        "mb": col128(np.asarray(inputs["match_b"], f32)),
        "wgT": np.ascontiguousarray(np.asarray(inputs["geom_w"], f32).T),
        "gb": col128(np.asarray(inputs["geom_b"], f32)),
        "nmw": np.asarray(inputs["normm_w"], f32).reshape(1, -1),
        "nmb": np.asarray(inputs["normm_b"], f32).reshape(1, -1),
        "ngw": np.asarray(inputs["normg_w"], f32).reshape(1, -1),
        "ngb": np.asarray(inputs["normg_b"], f32).reshape(1, -1),
    }


def kernel(**inputs):
    from concourse import bass_utils
    if "nc" not in _cache:
        _cache["nc"] = _build()
    nc = _cache["nc"]
    in_maps = [_host_inputs(inputs, c) for c in range(8)]
    res = bass_utils.run_bass_kernel_spmd(nc, in_maps, core_ids=list(range(8)),
                                          trace=bool(os.environ.get("BASS_TRACE")))
    _cache["last"] = res
    if res.exec_time_ns is not None:
        print(f"HW exec time: {res.exec_time_ns} ns")
        if res.instructions_and_trace:
            print("trace:", res.instructions_and_trace[1])
    ym = np.zeros((B, L, DIM), np.float32)
    yg = np.zeros((B, L, D_GEOM), np.float32)
    for b in range(B):
        ym[b] = res.results[2 * b]["ym"]
        yg[b] = res.results[2 * b]["yg"]
    return ym, yg
